# revision 36
# baseline (speedup 1.0000x reference)
"""2-layer GCN (gnn_message_passing) on 8 Trainium2 NeuronCores.

Strategy (graph/data parallel, dst-sharded, three SPMD launches):
  - Nodes sharded across 8 cores by destination id (12500 each). Host
    precomputes symmetric GCN normalization (graph preprocessing), adds
    self-loops, and bin-packs each core's nodes into uniform "chunks"
    (<=8 nodes, <=128 in-edges) with a target-chasing packer that fills
    chunks to ~97% of the 128-lane capacity. One NEFF per stage runs
    SPMD on all 8 cores with per-core metadata tensors.
  - No GPSIMD gather ucode in this image, so per-edge random gather is
    done by the host: it materializes the per-edge source-feature
    streams (the "gathered source features" of the halo exchange) in
    chunk layout; the device streams them and does all model math
    (transforms, aggregation matmuls, bias/relu, log_softmax) on-chip.
  - The wall clock is dominated by host->device transfer (axon tunnel,
    ~36 MB/s serial), so every stream byte counts:
    * transform-first: launch 0 computes xw1 = x @ W1 on device, so
      the layer-1 stream is 64 B/edge fp8 instead of 512 B/edge f32
      raw-x rows; the layer-2 stream is 40 B/edge fp8.
    * the norm weight w is folded into the fp8 stream rows on the host
      (pure data movement), so the device aggregates with 0/1 masks
      and no per-lane metadata at all.
    * per-chunk slot boundaries (9 fp16 values per chunk) are the only
      aggregation metadata; the device expands them to the 0/1 slot
      masks via a DMA partition-broadcast + DVE is_le/sub ops.
    * end-to-end max rel err ~7e-3 (fp32 PSUM accumulation) vs the
      2e-2 gate.
  - Launch A: per-chunk matmul msg^T @ mask -> feature-major PSUM,
    fused b1+ReLU (scalar engine), W2 matmul -> xw2 shard [slots, 40]
    fp8 (no transpose needed: W2 matmul emits slot-major directly).
  - Host halo exchange: concatenates xw2 shards, gathers the layer-2
    per-edge stream by source position, folding in w.
  - Launch B: aggregate the same way, PE-transpose to slot-major,
    add b2, log_softmax per node, write fp16 [slots, 40] per core.
  - Host un-permutes slot rows back to original node order.
"""

import numpy as np
import ml_dtypes

FULL = dict(N=100000, E=1600000, DIN=128, DH=64, DOUT=40)
CORES = 8
WSLOT = 8          # node slots per chunk
CHUNK = 128        # edge lanes per chunk
GRP = 16           # chunks per group  (GRP*WSLOT = 128 psum positions)
NP_F8 = ml_dtypes.float8_e4m3


# ------------------------------------------------------- host preprocessing
def _pack(degl):
    """Target-chasing bin-pack: <=WSLOT nodes, <=CHUNK edges per chunk.

    First item is the largest remaining degree; each further slot takes
    the available degree closest to cap/slots_left so chunks land near
    exactly CHUNK edges with ~WSLOT nodes (measured fill ~0.97).
    """
    n = len(degl)
    dmax = int(degl.max())
    by_deg = np.argsort(degl, kind="stable")
    startd = np.searchsorted(degl[by_deg], np.arange(dmax + 2))
    ptr = startd[1:].copy()              # pop position per degree bucket
    remaining = (startd[1:] - startd[:-1]).astype(np.int64)
    co = np.empty(n, np.int64)
    so = np.empty(n, np.int64)
    total, ci = n, 0
    while total > 0:
        # first: largest available
        d = dmax
        while d > 0 and remaining[d] == 0:
            d -= 1
        ptr[d] -= 1
        nl = by_deg[ptr[d]]
        remaining[d] -= 1
        total -= 1
        co[nl], so[nl] = ci, 0
        cap, k = CHUNK - d, 1
        while k < WSLOT and total > 0 and cap > 0:
            best, bestkey = 0, None
            if k == WSLOT - 2 and cap >= 2:
                # exact pair completion: pick d so that cap-d is also
                # available; the last slot then fills the chunk to CHUNK
                for dd in range(max(1, cap - dmax), min(cap - 1, dmax) + 1):
                    d2 = cap - dd
                    if d2 < 1 or d2 > dmax:
                        continue
                    if remaining[dd] >= (2 if d2 == dd else 1) \
                            and remaining[d2] >= 1:
                        key = (abs(dd - cap / 2), -dd)
                        if bestkey is None or key < bestkey:
                            bestkey, best = key, dd
            if best == 0:
                tgt = cap / (WSLOT - k)
                for d in range(1, min(cap, dmax) + 1):
                    if remaining[d] == 0:
                        continue
                    key = (abs(d - tgt), -d)
                    if bestkey is None or key < bestkey:
                        bestkey, best = key, d
            if best == 0:
                break
            ptr[best] -= 1
            nl = by_deg[ptr[best]]
            remaining[best] -= 1
            total -= 1
            co[nl], so[nl] = ci, k
            cap -= best
            k += 1
        ci += 1
    return co, so, ci


def preprocess(edge_index, cfg):
    """Graph preprocessing: norm weights, sharding, chunk packing.

    Returns per-core src ids / norm weight per edge lane
    ([CORES, CHUNK, c1]), per-chunk slot boundaries ([CORES, c1, 9]),
    slot maps, and the uniform chunk count c1.
    """
    N, NSH = cfg["N"], cfg["N"] // CORES
    src = np.asarray(edge_index[0], dtype=np.int64)
    dst = np.asarray(edge_index[1], dtype=np.int64)
    loops = np.arange(N, dtype=np.int64)
    s_all = np.concatenate([src, loops])
    d_all = np.concatenate([dst, loops])
    deg = np.bincount(d_all, minlength=N)
    dis = np.where(deg > 0, 1.0 / np.sqrt(np.maximum(deg, 1.0)), 0.0)
    dis = dis.astype(np.float32)

    o = np.argsort(d_all, kind="stable")
    s_srt, d_srt = s_all[o], d_all[o]
    w_srt = dis[s_srt] * dis[d_srt]
    seg = np.zeros(N + 1, np.int64)
    seg[1:] = np.cumsum(deg)

    # snake-balanced dst sharding: deal degree-sorted nodes across cores
    # so every core gets a near-identical degree multiset (equal packing)
    order = np.argsort(-deg, kind="stable")
    idx = np.arange(N)
    pos = idx % CORES
    core_rank = np.where((idx // CORES) % 2 == 0, pos, CORES - 1 - pos)
    nodes = [order[core_rank == c] for c in range(CORES)]

    packres = []
    nch = np.zeros(CORES, np.int64)
    for c in range(CORES):
        degl = deg[nodes[c]]
        assert degl.max() <= CHUNK, "node degree exceeds chunk capacity"
        assert degl.min() >= 1
        co, so, ncc = _pack(degl)
        packres.append((co, so))
        nch[c] = ncc

    c1 = ((int(nch.max()) + GRP - 1) // GRP) * GRP
    slots = c1 * WSLOT

    pos_of = np.empty(N, np.int64)
    rowpos = np.empty(N, np.int64)
    slot2node = np.full((CORES, slots), -1, np.int64)
    srcs = np.zeros((CORES, CHUNK, c1), np.int64)
    wml = np.zeros((CORES, CHUNK, c1), np.float16)
    bnd = np.zeros((CORES, c1, WSLOT + 1), np.float16)

    for c in range(CORES):
        ndc = nodes[c]
        co, so = packres[c]
        degl = deg[ndc]
        # lane base per node: exclusive cumsum of degrees in (chunk, slot) order
        ordk = np.argsort(co * WSLOT + so)
        degk = degl[ordk]
        cs = np.cumsum(degk) - degk
        cid = co[ordk]
        first = np.searchsorted(cid, np.arange(nch[c]), side="left")
        lane_base = np.empty(NSH, np.int64)
        lane_base[ordk] = cs - cs[first][cid]
        # ragged expansion of this core's edges (dst-sorted rows per node)
        lens = degl
        tot = int(lens.sum())
        cum = np.cumsum(lens) - lens
        within = np.arange(tot) - np.repeat(cum, lens)
        rows = np.repeat(seg[ndc], lens) + within
        eloc = np.repeat(np.arange(NSH), lens)
        lane_e = lane_base[eloc] + within
        assert lane_e.max() < CHUNK
        srcs[c, lane_e, co[eloc]] = s_srt[rows]
        wml[c, lane_e, co[eloc]] = w_srt[rows]
        pos_of[ndc] = c * slots + co * WSLOT + so
        rowpos[ndc] = c * NSH + np.arange(NSH)
        slot2node[c, co * WSLOT + so] = ndc
        # per-chunk slot boundaries: bnd[ci, s] = first lane of slot s,
        # bnd[ci, 8] = chunk fill; empty slots / pad chunks collapse to fill
        fill = np.zeros(c1, np.int64)
        np.add.at(fill, co, degl)
        bnd[c] = np.repeat(fill[:, None], WSLOT + 1, axis=1)
        bnd[c, co, so] = lane_base

    return dict(srcs=srcs, wml=wml, bnd=bnd, slot2node=slot2node,
                pos_of=pos_of, rowpos=rowpos, nodes=nodes, c1=c1, slots=slots)


def build_es(table, refs, wml, c1):
    """Gather per-edge rows, fold in the norm weight, emit fp8 stream.

    Returns [ng, CHUNK, GRP, width] = fp8(w_lane * table[refs]) so the
    device aggregates with a pure 0/1 slot mask.
    """
    ng = c1 // GRP
    r = refs.reshape(CHUNK, ng, GRP).transpose(1, 0, 2)
    wr = wml.astype(np.float32).reshape(CHUNK, ng, GRP).transpose(1, 0, 2)
    return (table[r].astype(np.float32) * wr[..., None]).astype(NP_F8)


# ------------------------------------------------------- numpy emulation
def emulate(x, W1, b1, W2, b2, meta, cfg):
    """Pure-numpy emulation of the device dataflow (logic validation)."""
    DOUT = cfg["DOUT"]
    c1, slots = meta["c1"], meta["slots"]
    srcs = meta["srcs"]
    lane = np.arange(CHUNK, dtype=np.float32)
    ge = meta["bnd"].astype(np.float32)[:, None, :, :] <= \
        lane[None, :, None, None]                 # [CORES, CHUNK, c1, 9]
    oh = ge[..., :WSLOT].astype(np.float32) - ge[..., 1:].astype(np.float32)
    wf = meta["wml"].astype(np.float32)[..., None]
    xw1 = x @ W1
    xw2_all = np.zeros((CORES * slots, DOUT), np.float32)
    for c in range(CORES):
        msg = wf[c] * xw1[srcs[c]]                # [CHUNK, c1, DH]
        hrawT = np.einsum("pcf,pcs->fcs", msg, oh[c]).reshape(-1, slots)
        hT = np.maximum(hrawT + b1[:, None], 0.0)
        xw2_all[c * slots:(c + 1) * slots] = (W2.T @ hT).T
    out_full = np.zeros((cfg["N"], DOUT), np.float32)
    for c in range(CORES):
        msg = wf[c] * xw2_all[meta["pos_of"][srcs[c]]]
        oT = np.einsum("pcf,pcs->fcs", msg, oh[c]).reshape(DOUT, slots)
        o = oT.T + b2[None, :]
        m = o.max(axis=1, keepdims=True)
        ls = (o - m) - np.log(np.exp(o - m).sum(axis=1, keepdims=True))
        sel = meta["slot2node"][c] >= 0
        out_full[meta["slot2node"][c][sel]] = ls[sel]
    return out_full


# ------------------------------------------------------- bass programs
def _bass_mods():
    import concourse.bass as bass
    import concourse.bacc as bacc
    import concourse.mybir as mybir
    import concourse.tile as tile
    return bass, bacc, mybir, tile


def _build_onehot(nc, tc, cpool, mybir, bnd_d, iota_d, c1):
    """Expand per-chunk slot boundaries to 0/1 masks [128, c1, 8].

    bnd[ci, s] is the first lane of slot s (bnd[ci, 8] = chunk fill);
    mask[lane, ci, s] = (bnd[s] <= lane < bnd[s+1]). Pad lanes and pad
    chunks fall outside every [bnd[s], bnd[s+1]) interval, so they are
    masked out structurally (their stream rows are zero as well).
    """
    F16 = mybir.dt.float16
    F32 = mybir.dt.float32
    NB = WSLOT + 1
    ALU = mybir.AluOpType
    iota_s = cpool.tile([CHUNK, 1], F32)
    nc.sync.dma_start(iota_s[:], iota_d[:].unsqueeze(1))
    oh = cpool.tile([CHUNK, c1, WSLOT], F16)
    with tc.tile_pool(name="ohtmp", bufs=1) as tpool:
        bb = tpool.tile([CHUNK, c1, NB], F16)
        nc.sync.dma_start(
            bb[:], bnd_d[:, :].unsqueeze(0).broadcast_to([CHUNK, c1, NB]))
        ge = tpool.tile([CHUNK, c1, NB], F16)
        for s in range(NB):
            nc.vector.tensor_scalar(ge[:, :, s], bb[:, :, s], iota_s[:], None,
                                    ALU.is_le)
        for s in range(WSLOT):
            nc.vector.tensor_tensor(oh[:, :, s], ge[:, :, s], ge[:, :, s + 1],
                                    ALU.subtract)
    return oh


def build_nc_0(cfg, nsh):
    """Launch 0: xw1 = x @ W1 per node shard (transform-first)."""
    bass, bacc, mybir, tile = _bass_mods()
    DIN, DH = cfg["DIN"], cfg["DH"]
    F8, F16, F32 = mybir.dt.float8e4, mybir.dt.float16, mybir.dt.float32
    PS = bass.MemorySpace.PSUM

    nc = bacc.Bacc(None, target_bir_lowering=False, num_devices=CORES)
    xT_d = nc.dram_tensor("xT", [DIN, nsh], F8, kind="ExternalInput")
    w1_d = nc.dram_tensor("W1", [DIN, DH], F16, kind="ExternalInput")
    xw1_d = nc.dram_tensor("xw1", [nsh, DH], F8, kind="ExternalOutput")

    with tile.TileContext(nc) as tc:
        with tc.tile_pool(name="const", bufs=1) as cpool:
            w1_s = cpool.tile([DIN, DH], F16)
            nc.sync.dma_start(w1_s[:], w1_d[:, :])
            xT_s = cpool.tile([DIN, nsh], F8)
            nc.sync.dma_start(xT_s[:], xT_d[:, :])
            with (
                tc.tile_pool(name="work", bufs=3) as wpool,
                tc.tile_pool(name="ps", bufs=4, space=PS) as pp,
            ):
                for t in range((nsh + 127) // 128):
                    n0 = t * 128
                    nn = min(128, nsh - n0)
                    p = pp.tile([128, DH], F32, tag="p")
                    nc.tensor.matmul(p[0:nn, :], xT_s[:, n0:n0 + nn],
                                     w1_s[:], start=True, stop=True)
                    ot = wpool.tile([128, DH], F8, tag="ot")
                    nc.vector.tensor_copy(ot[0:nn, :], p[0:nn, :])
                    nc.sync.dma_start(xw1_d[n0:n0 + nn, :], ot[0:nn, :])
    nc.compile()
    return nc


def build_nc_A(cfg, c1):
    """Launch A: layer-1 aggregation + b1/relu/W2 transform -> xw2 shard."""
    bass, bacc, mybir, tile = _bass_mods()
    DH, DOUT = cfg["DH"], cfg["DOUT"]
    F8, F16, F32 = mybir.dt.float8e4, mybir.dt.float16, mybir.dt.float32
    AF = mybir.ActivationFunctionType
    PS = bass.MemorySpace.PSUM
    slots, ng = c1 * WSLOT, c1 // GRP

    nc = bacc.Bacc(None, target_bir_lowering=False, num_devices=CORES)
    es_d = nc.dram_tensor("es", [ng, CHUNK, GRP, DH], F8, kind="ExternalInput")
    bnd_d = nc.dram_tensor("bnd", [c1, WSLOT + 1], F16, kind="ExternalInput")
    iota_d = nc.dram_tensor("iota", [CHUNK], F32, kind="ExternalInput")
    w2_d = nc.dram_tensor("W2", [DH, DOUT], F16, kind="ExternalInput")
    b1_d = nc.dram_tensor("b1", [DH], F32, kind="ExternalInput")
    xw2_d = nc.dram_tensor("xw2", [slots, DOUT], F8, kind="ExternalOutput")

    with tile.TileContext(nc) as tc:
        with tc.tile_pool(name="const", bufs=1) as cpool:
            w2_s = cpool.tile([DH, DOUT], F16)
            nc.sync.dma_start(w2_s[:], w2_d[:, :])
            b1_s = cpool.tile([DH, 1], F32)
            nc.sync.dma_start(b1_s[:], b1_d[:].unsqueeze(1))
            oh = _build_onehot(nc, tc, cpool, mybir, bnd_d, iota_d, c1)
            with (
                tc.tile_pool(name="gath", bufs=2) as gpool,
                tc.tile_pool(name="work", bufs=2) as wpool,
                tc.tile_pool(name="ps1", bufs=2, space=PS) as pp,
                tc.tile_pool(name="ps2", bufs=2, space=PS) as ppb,
            ):
                for g in range(ng):
                    msg = gpool.tile([CHUNK, GRP, DH], F8, tag="msg")
                    nc.sync.dma_start(msg[:], es_d[g, :, :, :])
                    pg = pp.tile([DH, GRP * WSLOT], F32, tag="agg")
                    for c in range(GRP):
                        nc.tensor.matmul(
                            pg[:, c * WSLOT:(c + 1) * WSLOT],
                            msg[:, c, :], oh[:, g * GRP + c, :],
                            start=True, stop=True)
                    hT = wpool.tile([DH, GRP * WSLOT], F16, tag="hT")
                    nc.scalar.activation(hT[:], pg[:], AF.Relu, bias=b1_s[:])
                    for k in range(GRP * WSLOT // 128):
                        p2 = ppb.tile([128, DOUT], F32, tag="p2")
                        nc.tensor.matmul(p2[:], hT[:, k * 128:(k + 1) * 128],
                                         w2_s[:], start=True, stop=True)
                        ot = wpool.tile([128, DOUT], F8, tag="ot")
                        nc.vector.tensor_copy(ot[:], p2[:])
                        r0 = (g * (GRP * WSLOT // 128) + k) * 128
                        nc.sync.dma_start(xw2_d[r0:r0 + 128, :], ot[:])
    nc.compile()
    return nc


def build_nc_B(cfg, c1):
    """Launch B: layer-2 aggregation + b2 + log_softmax -> output shard."""
    bass, bacc, mybir, tile = _bass_mods()
    DOUT = cfg["DOUT"]
    F8, F16, F32 = mybir.dt.float8e4, mybir.dt.float16, mybir.dt.float32
    AF = mybir.ActivationFunctionType
    ALU = mybir.AluOpType
    AX = mybir.AxisListType
    PS = bass.MemorySpace.PSUM
    slots, ng = c1 * WSLOT, c1 // GRP

    nc = bacc.Bacc(None, target_bir_lowering=False, num_devices=CORES)
    es_d = nc.dram_tensor("es", [ng, CHUNK, GRP, DOUT], F8, kind="ExternalInput")
    bnd_d = nc.dram_tensor("bnd", [c1, WSLOT + 1], F16, kind="ExternalInput")
    iota_d = nc.dram_tensor("iota", [CHUNK], F32, kind="ExternalInput")
    b2_d = nc.dram_tensor("b2", [DOUT], F32, kind="ExternalInput")
    id_d = nc.dram_tensor("ident", [DOUT, DOUT], F32, kind="ExternalInput")
    out_d = nc.dram_tensor("out", [slots, DOUT], F16, kind="ExternalOutput")

    with tile.TileContext(nc) as tc:
        with tc.tile_pool(name="const", bufs=1) as cpool:
            id_s = cpool.tile([DOUT, DOUT], F32)
            nc.sync.dma_start(id_s[:], id_d[:, :])
            b2r_s = cpool.tile([1, DOUT], F32)
            nc.sync.dma_start(b2r_s[:], b2_d[:].unsqueeze(0))
            ones_s = cpool.tile([1, 128], F32)
            nc.vector.memset(ones_s[:], 1.0)
            b2b_s = cpool.tile([128, DOUT], F32)
            with tc.tile_pool(name="pbc", bufs=1, space=PS) as pbc:
                pb = pbc.tile([128, DOUT], F32)
                nc.tensor.matmul(pb[:], ones_s[:], b2r_s[:], start=True, stop=True)
                nc.vector.tensor_copy(b2b_s[:], pb[:])
            oh = _build_onehot(nc, tc, cpool, mybir, bnd_d, iota_d, c1)
            with (
                tc.tile_pool(name="gath", bufs=2) as gpool,
                tc.tile_pool(name="work", bufs=2) as wpool,
                tc.tile_pool(name="ps1", bufs=2, space=PS) as pp,
                tc.tile_pool(name="ps2", bufs=2, space=PS) as ppb,
            ):
                for g in range(ng):
                    msg = gpool.tile([CHUNK, GRP, DOUT], F8, tag="msg")
                    nc.sync.dma_start(msg[:], es_d[g, :, :, :])
                    pg = pp.tile([DOUT, GRP * WSLOT], F32, tag="agg")
                    for c in range(GRP):
                        nc.tensor.matmul(
                            pg[:, c * WSLOT:(c + 1) * WSLOT],
                            msg[:, c, :], oh[:, g * GRP + c, :],
                            start=True, stop=True)
                    oT = wpool.tile([DOUT, GRP * WSLOT], F32, tag="oT")
                    nc.scalar.copy(oT[:], pg[:])
                    for k in range(GRP * WSLOT // 128):
                        pt = ppb.tile([128, DOUT], F32, tag="pt")
                        nc.tensor.transpose(pt[:], oT[:, k * 128:(k + 1) * 128],
                                            id_s[:])
                        t = wpool.tile([128, DOUT], F32, tag="t")
                        nc.vector.tensor_tensor(t[:], pt[:], b2b_s[:], ALU.add)
                        mx = wpool.tile([128, 1], F32, tag="mx")
                        nc.vector.tensor_reduce(mx[:], t[:], AX.X, ALU.max)
                        sh = wpool.tile([128, DOUT], F32, tag="sh")
                        nc.vector.tensor_scalar_sub(sh[:], t[:], mx[:])
                        ex = wpool.tile([128, DOUT], F32, tag="ex")
                        nc.scalar.activation(ex[:], sh[:], AF.Exp)
                        sm = wpool.tile([128, 1], F32, tag="sm")
                        nc.vector.tensor_reduce(sm[:], ex[:], AX.X, ALU.add)
                        lg = wpool.tile([128, 1], F32, tag="lg")
                        nc.scalar.activation(lg[:], sm[:], AF.Ln)
                        res = wpool.tile([128, DOUT], F16, tag="res")
                        nc.vector.tensor_scalar_sub(res[:], sh[:], lg[:])
                        r0 = (g * (GRP * WSLOT // 128) + k) * 128
                        nc.sync.dma_start(out_d[r0:r0 + 128, :], res[:])
    nc.compile()
    return nc


# ------------------------------------------------------- public entry
def kernel(x, edge_index, W1, b1, W2, b2, cfg=None, trace=False, time_reps=0):
    import time as _time

    from concourse.bass_utils import run_bass_kernel_spmd

    cfg = cfg or FULL
    N, NSH = cfg["N"], cfg["N"] // CORES
    DIN, DH, DOUT = cfg["DIN"], cfg["DH"], cfg["DOUT"]
    x = np.ascontiguousarray(np.asarray(x, dtype=np.float32))
    W1_h = np.asarray(W1, dtype=np.float32).astype(np.float16)
    b1_h = np.asarray(b1, dtype=np.float32)
    W2_h = np.asarray(W2, dtype=np.float32).astype(np.float16)
    b2_h = np.asarray(b2, dtype=np.float32)
    ident = np.eye(DOUT, dtype=np.float32)

    meta = preprocess(edge_index, cfg)
    c1, slots = meta["c1"], meta["slots"]

    def timed(nc, ins, store):
        res = run_bass_kernel_spmd(nc, ins, core_ids=list(range(CORES)),
                                   trace=trace)
        for _ in range(time_reps):
            t0 = _time.perf_counter()
            run_bass_kernel_spmd(nc, ins, core_ids=list(range(CORES)))
            store.append(_time.perf_counter() - t0)
        return res

    # ---- launch 0: xw1 = x @ W1 ----
    xq = x.astype(NP_F8)
    xT_in = [np.ascontiguousarray(xq[meta["nodes"][c]].T)
             for c in range(CORES)]
    nc_0 = build_nc_0(cfg, NSH)
    in_0 = [{"xT": xT_in[c], "W1": W1_h} for c in range(CORES)]
    kernel.times_0 = []
    res_0 = timed(nc_0, in_0, kernel.times_0)
    xw1_all = np.concatenate(
        [res_0.results[c]["xw1"] for c in range(CORES)], 0)

    # ---- launch A: layer 1 ----
    ref1 = meta["rowpos"][meta["srcs"]]          # xw1_all rows per lane
    lane_iota = np.arange(CHUNK, dtype=np.float32)
    nc_a = build_nc_A(cfg, c1)
    in_a = [{"es": build_es(xw1_all, ref1[c], meta["wml"][c], c1),
             "bnd": meta["bnd"][c], "iota": lane_iota,
             "W2": W2_h, "b1": b1_h} for c in range(CORES)]
    kernel.times_a = []
    res_a = timed(nc_a, in_a, kernel.times_a)

    # ---- host halo exchange ----
    xw2_all = np.concatenate(
        [res_a.results[c]["xw2"] for c in range(CORES)], 0)
    ref2 = meta["pos_of"][meta["srcs"]]          # [CORES, CHUNK, c1]

    # ---- launch B: layer 2 ----
    nc_b = build_nc_B(cfg, c1)
    in_b = [{"es": build_es(xw2_all, ref2[c], meta["wml"][c], c1),
             "bnd": meta["bnd"][c], "iota": lane_iota,
             "b2": b2_h, "ident": ident} for c in range(CORES)]
    kernel.times_b = []
    res_b = timed(nc_b, in_b, kernel.times_b)

    out_full = np.zeros((N, DOUT), np.float32)
    for c in range(CORES):
        o = res_b.results[c]["out"].astype(np.float32)
        sel = meta["slot2node"][c] >= 0
        out_full[meta["slot2node"][c][sel]] = o[sel]
    return out_full


if __name__ == "__main__":
    cfg = dict(N=4096, E=65536, DIN=128, DH=64, DOUT=40)
    rng = np.random.default_rng(0)
    x = rng.normal(size=(cfg["N"], cfg["DIN"])).astype(np.float32)
    ei = rng.integers(0, cfg["N"], size=(2, cfg["E"])).astype(np.int64)
    W1 = (rng.normal(size=(cfg["DIN"], cfg["DH"])) / 16).astype(np.float32)
    b1 = (rng.normal(size=(cfg["DH"],)) * 0.1).astype(np.float32)
    W2 = (rng.normal(size=(cfg["DH"], cfg["DOUT"])) / 8).astype(np.float32)
    b2 = (rng.normal(size=(cfg["DOUT"],)) * 0.1).astype(np.float32)

    meta = preprocess(ei, cfg)
    print("c1:", meta["c1"], "slots:", meta["slots"],
          "pack_eff:", (cfg["E"] + cfg["N"]) / (meta["c1"] * CHUNK * CORES))
    got = emulate(x, W1, b1, W2, b2, meta, cfg)

    N = cfg["N"]
    loops = np.arange(N, dtype=np.int64)
    s = np.concatenate([ei[0], loops]); d = np.concatenate([ei[1], loops])
    deg = np.bincount(d, minlength=N).astype(np.float32)
    dis = np.where(deg > 0, 1 / np.sqrt(np.maximum(deg, 1)), 0).astype(np.float32)
    w = dis[s] * dis[d]

    def conv(xx, W, b):
        xw = xx @ W
        out = np.zeros((N, W.shape[1]), dtype=np.float32)
        np.add.at(out, d, xw[s] * w[:, None])
        return out + b

    h = np.maximum(conv(x, W1, b1), 0)
    o = conv(h, W2, b2)
    m = o.max(1, keepdims=True)
    ref = (o - m) - np.log(np.exp(o - m).sum(1, keepdims=True))
    err = np.abs(got - ref).max() / (np.abs(ref).max() + 1e-9)
    print("emulator vs ref max rel err:", err)
    assert err < 2e-3, err
    print("HOST LOGIC OK")


# revision 37
# speedup vs baseline: 1.0797x; 1.0797x over previous
"""2-layer GCN (gnn_message_passing) on 8 Trainium2 NeuronCores.

Strategy (graph/data parallel, dst-sharded, three SPMD launches):
  - Nodes sharded across 8 cores by destination id (12500 each). Host
    precomputes symmetric GCN normalization (graph preprocessing), adds
    self-loops, and bin-packs each core's nodes into uniform "chunks"
    (<=8 nodes, <=128 in-edges). Snake-balanced sharding (degree-sorted
    deal) plus a target-chasing packer with exact pair completion fill
    chunks to ~98% of the 128-lane capacity. One NEFF per stage runs
    SPMD on all 8 cores with per-core metadata tensors.
  - No GPSIMD gather ucode in this image, so per-edge random gather is
    done by the host: it materializes the per-edge source-feature
    streams (the "gathered source features" of the halo exchange) in
    chunk layout; the device streams them and does all model math
    (transforms, aggregation matmuls, bias/relu, log_softmax) on-chip.
  - The wall clock is dominated by host->device transfer (axon tunnel,
    ~36 MB/s serial), so every stream byte counts:
    * transform-first: launch 0 computes xw1 = x @ W1 on device, so
      the layer-1 stream is 64 B/edge fp8 instead of 512 B/edge f32
      raw-x rows; the layer-2 stream is 40 B/edge fp8.
    * the norm weight w is folded into the fp8 stream rows on the host
      (pure data movement), so the device aggregates with 0/1 masks
      and no per-lane metadata at all.
    * per-chunk slot boundaries (9 fp16 values per chunk) are the only
      aggregation metadata; the device expands them to the 0/1 slot
      masks via a DMA partition-broadcast + DVE is_le/sub ops.
    * end-to-end max rel err ~7e-3 (fp32 PSUM accumulation) vs the
      2e-2 gate.
  - Launch A: per-chunk matmul msg^T @ mask -> feature-major PSUM,
    fused b1+ReLU (scalar engine), W2 matmul -> xw2 shard [slots, 40]
    fp8 (no transpose needed: W2 matmul emits slot-major directly).
  - Host halo exchange: concatenates xw2 shards, gathers the layer-2
    per-edge stream by source position, folding in w.
  - Launch B: aggregate the same way, PE-transpose to slot-major,
    add b2, log_softmax per node, write fp16 [slots, 40] per core.
  - Host un-permutes slot rows back to original node order.
"""

import numpy as np
import ml_dtypes

FULL = dict(N=100000, E=1600000, DIN=128, DH=64, DOUT=40)
CORES = 8
WSLOT = 8          # node slots per chunk
CHUNK = 128        # edge lanes per chunk
GRP = 16           # chunks per group  (GRP*WSLOT = 128 psum positions)
NP_F8 = ml_dtypes.float8_e4m3


# ------------------------------------------------------- host preprocessing
def _pack(degl):
    """Target-chasing bin-pack: <=WSLOT nodes, <=CHUNK edges per chunk.

    First item is the largest remaining degree; each further slot takes
    the available degree closest to cap/slots_left so chunks land near
    exactly CHUNK edges with ~WSLOT nodes (measured fill ~0.97).
    """
    n = len(degl)
    dmax = int(degl.max())
    by_deg = np.argsort(degl, kind="stable")
    startd = np.searchsorted(degl[by_deg], np.arange(dmax + 2))
    ptr = startd[1:].copy()              # pop position per degree bucket
    remaining = (startd[1:] - startd[:-1]).astype(np.int64)
    co = np.empty(n, np.int64)
    so = np.empty(n, np.int64)
    total, ci = n, 0
    while total > 0:
        # first: largest available
        d = dmax
        while d > 0 and remaining[d] == 0:
            d -= 1
        ptr[d] -= 1
        nl = by_deg[ptr[d]]
        remaining[d] -= 1
        total -= 1
        co[nl], so[nl] = ci, 0
        cap, k = CHUNK - d, 1
        while k < WSLOT and total > 0 and cap > 0:
            best, bestkey = 0, None
            if k == WSLOT - 2 and cap >= 2:
                # exact pair completion: pick d so that cap-d is also
                # available; the last slot then fills the chunk to CHUNK
                for dd in range(max(1, cap - dmax), min(cap - 1, dmax) + 1):
                    d2 = cap - dd
                    if d2 < 1 or d2 > dmax:
                        continue
                    if remaining[dd] >= (2 if d2 == dd else 1) \
                            and remaining[d2] >= 1:
                        key = (abs(dd - cap / 2), -dd)
                        if bestkey is None or key < bestkey:
                            bestkey, best = key, dd
            if best == 0:
                tgt = cap / (WSLOT - k)
                for d in range(1, min(cap, dmax) + 1):
                    if remaining[d] == 0:
                        continue
                    key = (abs(d - tgt), -d)
                    if bestkey is None or key < bestkey:
                        bestkey, best = key, d
            if best == 0:
                break
            ptr[best] -= 1
            nl = by_deg[ptr[best]]
            remaining[best] -= 1
            total -= 1
            co[nl], so[nl] = ci, k
            cap -= best
            k += 1
        ci += 1
    return co, so, ci


def preprocess(edge_index, cfg):
    """Graph preprocessing: norm weights, sharding, chunk packing.

    Returns per-core src ids / norm weight per edge lane
    ([CORES, CHUNK, c1]), per-chunk slot boundaries ([CORES, c1, 9]),
    slot maps, and the uniform chunk count c1.
    """
    N, NSH = cfg["N"], cfg["N"] // CORES
    src = np.asarray(edge_index[0], dtype=np.int64)
    dst = np.asarray(edge_index[1], dtype=np.int64)
    loops = np.arange(N, dtype=np.int64)
    s_all = np.concatenate([src, loops])
    d_all = np.concatenate([dst, loops])
    deg = np.bincount(d_all, minlength=N)
    dis = np.where(deg > 0, 1.0 / np.sqrt(np.maximum(deg, 1.0)), 0.0)
    dis = dis.astype(np.float32)

    o = np.argsort(d_all, kind="stable")
    s_srt, d_srt = s_all[o], d_all[o]
    w_srt = dis[s_srt] * dis[d_srt]
    seg = np.zeros(N + 1, np.int64)
    seg[1:] = np.cumsum(deg)

    # snake-balanced dst sharding: deal degree-sorted nodes across cores
    # so every core gets a near-identical degree multiset (equal packing)
    order = np.argsort(-deg, kind="stable")
    idx = np.arange(N)
    pos = idx % CORES
    core_rank = np.where((idx // CORES) % 2 == 0, pos, CORES - 1 - pos)
    nodes = [order[core_rank == c] for c in range(CORES)]

    packres = []
    nch = np.zeros(CORES, np.int64)
    for c in range(CORES):
        degl = deg[nodes[c]]
        assert degl.max() <= CHUNK, "node degree exceeds chunk capacity"
        assert degl.min() >= 1
        co, so, ncc = _pack(degl)
        packres.append((co, so))
        nch[c] = ncc

    c1 = ((int(nch.max()) + GRP - 1) // GRP) * GRP
    slots = c1 * WSLOT

    pos_of = np.empty(N, np.int64)
    rowpos = np.empty(N, np.int64)
    slot2node = np.full((CORES, slots), -1, np.int64)
    srcs = np.zeros((CORES, CHUNK, c1), np.int64)
    wml = np.zeros((CORES, CHUNK, c1), np.float16)
    bnd = np.zeros((CORES, c1, WSLOT + 1), np.float16)

    for c in range(CORES):
        ndc = nodes[c]
        co, so = packres[c]
        degl = deg[ndc]
        # lane base per node: exclusive cumsum of degrees in (chunk, slot) order
        ordk = np.argsort(co * WSLOT + so)
        degk = degl[ordk]
        cs = np.cumsum(degk) - degk
        cid = co[ordk]
        first = np.searchsorted(cid, np.arange(nch[c]), side="left")
        lane_base = np.empty(NSH, np.int64)
        lane_base[ordk] = cs - cs[first][cid]
        # ragged expansion of this core's edges (dst-sorted rows per node)
        lens = degl
        tot = int(lens.sum())
        cum = np.cumsum(lens) - lens
        within = np.arange(tot) - np.repeat(cum, lens)
        rows = np.repeat(seg[ndc], lens) + within
        eloc = np.repeat(np.arange(NSH), lens)
        lane_e = lane_base[eloc] + within
        assert lane_e.max() < CHUNK
        srcs[c, lane_e, co[eloc]] = s_srt[rows]
        wml[c, lane_e, co[eloc]] = w_srt[rows]
        pos_of[ndc] = c * slots + co * WSLOT + so
        rowpos[ndc] = c * NSH + np.arange(NSH)
        slot2node[c, co * WSLOT + so] = ndc
        # per-chunk slot boundaries: bnd[ci, s] = first lane of slot s,
        # bnd[ci, 8] = chunk fill; empty slots / pad chunks collapse to fill
        fill = np.zeros(c1, np.int64)
        np.add.at(fill, co, degl)
        bnd[c] = np.repeat(fill[:, None], WSLOT + 1, axis=1)
        bnd[c, co, so] = lane_base

    return dict(srcs=srcs, wml=wml, bnd=bnd, slot2node=slot2node,
                pos_of=pos_of, rowpos=rowpos, nodes=nodes, c1=c1, slots=slots)


def build_es(table, refs, wml, c1):
    """Gather per-edge rows, fold in the norm weight, emit fp8 stream.

    Returns [ng, CHUNK, GRP, width] = fp8(w_lane * table[refs]) so the
    device aggregates with a pure 0/1 slot mask.
    """
    ng = c1 // GRP
    r = refs.reshape(CHUNK, ng, GRP).transpose(1, 0, 2)
    wr = wml.astype(np.float32).reshape(CHUNK, ng, GRP).transpose(1, 0, 2)
    return (table[r].astype(np.float32) * wr[..., None]).astype(NP_F8)


# ------------------------------------------------------- numpy emulation
def emulate(x, W1, b1, W2, b2, meta, cfg):
    """Pure-numpy emulation of the device dataflow (logic validation)."""
    DOUT = cfg["DOUT"]
    c1, slots = meta["c1"], meta["slots"]
    srcs = meta["srcs"]
    lane = np.arange(CHUNK, dtype=np.float32)
    ge = meta["bnd"].astype(np.float32)[:, None, :, :] <= \
        lane[None, :, None, None]                 # [CORES, CHUNK, c1, 9]
    oh = ge[..., :WSLOT].astype(np.float32) - ge[..., 1:].astype(np.float32)
    wf = meta["wml"].astype(np.float32)[..., None]
    xw1 = x @ W1
    xw2_all = np.zeros((CORES * slots, DOUT), np.float32)
    for c in range(CORES):
        msg = wf[c] * xw1[srcs[c]]                # [CHUNK, c1, DH]
        hrawT = np.einsum("pcf,pcs->fcs", msg, oh[c]).reshape(-1, slots)
        hT = np.maximum(hrawT + b1[:, None], 0.0)
        xw2_all[c * slots:(c + 1) * slots] = (W2.T @ hT).T
    out_full = np.zeros((cfg["N"], DOUT), np.float32)
    for c in range(CORES):
        msg = wf[c] * xw2_all[meta["pos_of"][srcs[c]]]
        oT = np.einsum("pcf,pcs->fcs", msg, oh[c]).reshape(DOUT, slots)
        o = oT.T + b2[None, :]
        m = o.max(axis=1, keepdims=True)
        ls = (o - m) - np.log(np.exp(o - m).sum(axis=1, keepdims=True))
        sel = meta["slot2node"][c] >= 0
        out_full[meta["slot2node"][c][sel]] = ls[sel]
    return out_full


# ------------------------------------------------------- bass programs
def _bass_mods():
    import concourse.bass as bass
    import concourse.bacc as bacc
    import concourse.mybir as mybir
    import concourse.tile as tile
    return bass, bacc, mybir, tile


def _build_onehot(nc, tc, cpool, mybir, bnd_d, iota_d, c1):
    """Expand per-chunk slot boundaries to 0/1 masks [128, c1, 8].

    bnd[ci, s] is the first lane of slot s (bnd[ci, 8] = chunk fill);
    mask[lane, ci, s] = (bnd[s] <= lane < bnd[s+1]). Pad lanes and pad
    chunks fall outside every [bnd[s], bnd[s+1]) interval, so they are
    masked out structurally (their stream rows are zero as well).
    """
    F16 = mybir.dt.float16
    F32 = mybir.dt.float32
    NB = WSLOT + 1
    ALU = mybir.AluOpType
    iota_s = cpool.tile([CHUNK, 1], F32)
    nc.sync.dma_start(iota_s[:], iota_d[:].unsqueeze(1))
    oh = cpool.tile([CHUNK, c1, WSLOT], F16)
    with tc.tile_pool(name="ohtmp", bufs=1) as tpool:
        bb = tpool.tile([CHUNK, c1, NB], F16)
        nc.sync.dma_start(
            bb[:], bnd_d[:, :].unsqueeze(0).broadcast_to([CHUNK, c1, NB]))
        ge = tpool.tile([CHUNK, c1, NB], F16)
        for s in range(NB):
            nc.vector.tensor_scalar(ge[:, :, s], bb[:, :, s], iota_s[:], None,
                                    ALU.is_le)
        for s in range(WSLOT):
            nc.vector.tensor_tensor(oh[:, :, s], ge[:, :, s], ge[:, :, s + 1],
                                    ALU.subtract)
    return oh


def build_nc_0(cfg, nsh):
    """Launch 0: xw1 = x @ W1 per node shard (transform-first)."""
    bass, bacc, mybir, tile = _bass_mods()
    DIN, DH = cfg["DIN"], cfg["DH"]
    F8, F16, F32 = mybir.dt.float8e4, mybir.dt.float16, mybir.dt.float32
    PS = bass.MemorySpace.PSUM

    nc = bacc.Bacc(None, target_bir_lowering=False, num_devices=CORES)
    xT_d = nc.dram_tensor("xT", [DIN, nsh], F8, kind="ExternalInput")
    w1_d = nc.dram_tensor("W1", [DIN, DH], F16, kind="ExternalInput")
    xw1_d = nc.dram_tensor("xw1", [nsh, DH], F8, kind="ExternalOutput")

    with tile.TileContext(nc) as tc:
        with tc.tile_pool(name="const", bufs=1) as cpool:
            w1_s = cpool.tile([DIN, DH], F16)
            nc.sync.dma_start(w1_s[:], w1_d[:, :])
            xT_s = cpool.tile([DIN, nsh], F8)
            nc.sync.dma_start(xT_s[:], xT_d[:, :])
            with (
                tc.tile_pool(name="work", bufs=3) as wpool,
                tc.tile_pool(name="ps", bufs=4, space=PS) as pp,
            ):
                for t in range((nsh + 127) // 128):
                    n0 = t * 128
                    nn = min(128, nsh - n0)
                    p = pp.tile([128, DH], F32, tag="p")
                    nc.tensor.matmul(p[0:nn, :], xT_s[:, n0:n0 + nn],
                                     w1_s[:], start=True, stop=True)
                    ot = wpool.tile([128, DH], F8, tag="ot")
                    nc.vector.tensor_copy(ot[0:nn, :], p[0:nn, :])
                    nc.sync.dma_start(xw1_d[n0:n0 + nn, :], ot[0:nn, :])
    nc.compile()
    return nc


def build_nc_A(cfg, c1):
    """Launch A: layer-1 aggregation + b1/relu/W2 transform -> xw2 shard."""
    bass, bacc, mybir, tile = _bass_mods()
    DH, DOUT = cfg["DH"], cfg["DOUT"]
    F8, F16, F32 = mybir.dt.float8e4, mybir.dt.float16, mybir.dt.float32
    AF = mybir.ActivationFunctionType
    PS = bass.MemorySpace.PSUM
    slots, ng = c1 * WSLOT, c1 // GRP

    nc = bacc.Bacc(None, target_bir_lowering=False, num_devices=CORES)
    es_d = nc.dram_tensor("es", [ng, CHUNK, GRP, DH], F8, kind="ExternalInput")
    bnd_d = nc.dram_tensor("bnd", [c1, WSLOT + 1], F16, kind="ExternalInput")
    iota_d = nc.dram_tensor("iota", [CHUNK], F32, kind="ExternalInput")
    w2_d = nc.dram_tensor("W2", [DH, DOUT], F16, kind="ExternalInput")
    b1_d = nc.dram_tensor("b1", [DH], F32, kind="ExternalInput")
    xw2_d = nc.dram_tensor("xw2", [slots, DOUT], F8, kind="ExternalOutput")

    with tile.TileContext(nc) as tc:
        with tc.tile_pool(name="const", bufs=1) as cpool:
            w2_s = cpool.tile([DH, DOUT], F16)
            nc.sync.dma_start(w2_s[:], w2_d[:, :])
            b1_s = cpool.tile([DH, 1], F32)
            nc.sync.dma_start(b1_s[:], b1_d[:].unsqueeze(1))
            oh = _build_onehot(nc, tc, cpool, mybir, bnd_d, iota_d, c1)
            with (
                tc.tile_pool(name="gath", bufs=2) as gpool,
                tc.tile_pool(name="work", bufs=2) as wpool,
                tc.tile_pool(name="ps1", bufs=2, space=PS) as pp,
                tc.tile_pool(name="ps2", bufs=2, space=PS) as ppb,
            ):
                for g in range(ng):
                    msg = gpool.tile([CHUNK, GRP, DH], F8, tag="msg")
                    nc.sync.dma_start(msg[:], es_d[g, :, :, :])
                    pg = pp.tile([DH, GRP * WSLOT], F32, tag="agg")
                    for c in range(GRP):
                        nc.tensor.matmul(
                            pg[:, c * WSLOT:(c + 1) * WSLOT],
                            msg[:, c, :], oh[:, g * GRP + c, :],
                            start=True, stop=True)
                    hT = wpool.tile([DH, GRP * WSLOT], F16, tag="hT")
                    nc.scalar.activation(hT[:], pg[:], AF.Relu, bias=b1_s[:])
                    for k in range(GRP * WSLOT // 128):
                        p2 = ppb.tile([128, DOUT], F32, tag="p2")
                        nc.tensor.matmul(p2[:], hT[:, k * 128:(k + 1) * 128],
                                         w2_s[:], start=True, stop=True)
                        ot = wpool.tile([128, DOUT], F8, tag="ot")
                        nc.vector.tensor_copy(ot[:], p2[:])
                        r0 = (g * (GRP * WSLOT // 128) + k) * 128
                        nc.sync.dma_start(xw2_d[r0:r0 + 128, :], ot[:])
    nc.compile()
    return nc


def build_nc_B(cfg, c1):
    """Launch B: layer-2 aggregation + b2 + log_softmax -> output shard."""
    bass, bacc, mybir, tile = _bass_mods()
    DOUT = cfg["DOUT"]
    F8, F16, F32 = mybir.dt.float8e4, mybir.dt.float16, mybir.dt.float32
    AF = mybir.ActivationFunctionType
    ALU = mybir.AluOpType
    AX = mybir.AxisListType
    PS = bass.MemorySpace.PSUM
    slots, ng = c1 * WSLOT, c1 // GRP

    nc = bacc.Bacc(None, target_bir_lowering=False, num_devices=CORES)
    es_d = nc.dram_tensor("es", [ng, CHUNK, GRP, DOUT], F8, kind="ExternalInput")
    bnd_d = nc.dram_tensor("bnd", [c1, WSLOT + 1], F16, kind="ExternalInput")
    iota_d = nc.dram_tensor("iota", [CHUNK], F32, kind="ExternalInput")
    b2_d = nc.dram_tensor("b2", [DOUT], F32, kind="ExternalInput")
    id_d = nc.dram_tensor("ident", [DOUT, DOUT], F32, kind="ExternalInput")
    out_d = nc.dram_tensor("out", [slots, DOUT], F16, kind="ExternalOutput")

    with tile.TileContext(nc) as tc:
        with tc.tile_pool(name="const", bufs=1) as cpool:
            id_s = cpool.tile([DOUT, DOUT], F32)
            nc.sync.dma_start(id_s[:], id_d[:, :])
            b2r_s = cpool.tile([1, DOUT], F32)
            nc.sync.dma_start(b2r_s[:], b2_d[:].unsqueeze(0))
            ones_s = cpool.tile([1, 128], F32)
            nc.vector.memset(ones_s[:], 1.0)
            b2b_s = cpool.tile([128, DOUT], F32)
            with tc.tile_pool(name="pbc", bufs=1, space=PS) as pbc:
                pb = pbc.tile([128, DOUT], F32)
                nc.tensor.matmul(pb[:], ones_s[:], b2r_s[:], start=True, stop=True)
                nc.vector.tensor_copy(b2b_s[:], pb[:])
            oh = _build_onehot(nc, tc, cpool, mybir, bnd_d, iota_d, c1)
            with (
                tc.tile_pool(name="gath", bufs=2) as gpool,
                tc.tile_pool(name="work", bufs=2) as wpool,
                tc.tile_pool(name="ps1", bufs=2, space=PS) as pp,
                tc.tile_pool(name="ps2", bufs=2, space=PS) as ppb,
            ):
                for g in range(ng):
                    msg = gpool.tile([CHUNK, GRP, DOUT], F8, tag="msg")
                    nc.sync.dma_start(msg[:], es_d[g, :, :, :])
                    pg = pp.tile([DOUT, GRP * WSLOT], F32, tag="agg")
                    for c in range(GRP):
                        nc.tensor.matmul(
                            pg[:, c * WSLOT:(c + 1) * WSLOT],
                            msg[:, c, :], oh[:, g * GRP + c, :],
                            start=True, stop=True)
                    oT = wpool.tile([DOUT, GRP * WSLOT], F32, tag="oT")
                    nc.scalar.copy(oT[:], pg[:])
                    for k in range(GRP * WSLOT // 128):
                        pt = ppb.tile([128, DOUT], F32, tag="pt")
                        nc.tensor.transpose(pt[:], oT[:, k * 128:(k + 1) * 128],
                                            id_s[:])
                        t = wpool.tile([128, DOUT], F32, tag="t")
                        nc.vector.tensor_tensor(t[:], pt[:], b2b_s[:], ALU.add)
                        mx = wpool.tile([128, 1], F32, tag="mx")
                        nc.vector.tensor_reduce(mx[:], t[:], AX.X, ALU.max)
                        sh = wpool.tile([128, DOUT], F32, tag="sh")
                        nc.vector.tensor_scalar_sub(sh[:], t[:], mx[:])
                        ex = wpool.tile([128, DOUT], F32, tag="ex")
                        nc.scalar.activation(ex[:], sh[:], AF.Exp)
                        sm = wpool.tile([128, 1], F32, tag="sm")
                        nc.vector.tensor_reduce(sm[:], ex[:], AX.X, ALU.add)
                        lg = wpool.tile([128, 1], F32, tag="lg")
                        nc.scalar.activation(lg[:], sm[:], AF.Ln)
                        res = wpool.tile([128, DOUT], F16, tag="res")
                        nc.vector.tensor_scalar_sub(res[:], sh[:], lg[:])
                        r0 = (g * (GRP * WSLOT // 128) + k) * 128
                        nc.sync.dma_start(out_d[r0:r0 + 128, :], res[:])
    nc.compile()
    return nc


# ------------------------------------------------------- public entry
def kernel(x, edge_index, W1, b1, W2, b2, cfg=None, trace=False, time_reps=0):
    import time as _time

    from concourse.bass_utils import run_bass_kernel_spmd

    cfg = cfg or FULL
    N, NSH = cfg["N"], cfg["N"] // CORES
    DIN, DH, DOUT = cfg["DIN"], cfg["DH"], cfg["DOUT"]
    x = np.ascontiguousarray(np.asarray(x, dtype=np.float32))
    W1_h = np.asarray(W1, dtype=np.float32).astype(np.float16)
    b1_h = np.asarray(b1, dtype=np.float32)
    W2_h = np.asarray(W2, dtype=np.float32).astype(np.float16)
    b2_h = np.asarray(b2, dtype=np.float32)
    ident = np.eye(DOUT, dtype=np.float32)

    meta = preprocess(edge_index, cfg)
    c1, slots = meta["c1"], meta["slots"]

    def timed(nc, ins, store):
        res = run_bass_kernel_spmd(nc, ins, core_ids=list(range(CORES)),
                                   trace=trace)
        for _ in range(time_reps):
            t0 = _time.perf_counter()
            run_bass_kernel_spmd(nc, ins, core_ids=list(range(CORES)))
            store.append(_time.perf_counter() - t0)
        return res

    # ---- launch 0: xw1 = x @ W1 ----
    xq = x.astype(NP_F8)
    xT_in = [np.ascontiguousarray(xq[meta["nodes"][c]].T)
             for c in range(CORES)]
    nc_0 = build_nc_0(cfg, NSH)
    in_0 = [{"xT": xT_in[c], "W1": W1_h} for c in range(CORES)]
    kernel.times_0 = []
    res_0 = timed(nc_0, in_0, kernel.times_0)
    xw1_all = np.concatenate(
        [res_0.results[c]["xw1"] for c in range(CORES)], 0)

    # ---- launch A: layer 1 ----
    ref1 = meta["rowpos"][meta["srcs"]]          # xw1_all rows per lane
    lane_iota = np.arange(CHUNK, dtype=np.float32)
    nc_a = build_nc_A(cfg, c1)
    in_a = [{"es": build_es(xw1_all, ref1[c], meta["wml"][c], c1),
             "bnd": meta["bnd"][c], "iota": lane_iota,
             "W2": W2_h, "b1": b1_h} for c in range(CORES)]
    kernel.times_a = []
    res_a = timed(nc_a, in_a, kernel.times_a)

    # ---- host halo exchange ----
    xw2_all = np.concatenate(
        [res_a.results[c]["xw2"] for c in range(CORES)], 0)
    ref2 = meta["pos_of"][meta["srcs"]]          # [CORES, CHUNK, c1]

    # ---- launch B: layer 2 ----
    nc_b = build_nc_B(cfg, c1)
    in_b = [{"es": build_es(xw2_all, ref2[c], meta["wml"][c], c1),
             "bnd": meta["bnd"][c], "iota": lane_iota,
             "b2": b2_h, "ident": ident} for c in range(CORES)]
    kernel.times_b = []
    res_b = timed(nc_b, in_b, kernel.times_b)

    out_full = np.zeros((N, DOUT), np.float32)
    for c in range(CORES):
        o = res_b.results[c]["out"].astype(np.float32)
        sel = meta["slot2node"][c] >= 0
        out_full[meta["slot2node"][c][sel]] = o[sel]
    return out_full


if __name__ == "__main__":
    cfg = dict(N=4096, E=65536, DIN=128, DH=64, DOUT=40)
    rng = np.random.default_rng(0)
    x = rng.normal(size=(cfg["N"], cfg["DIN"])).astype(np.float32)
    ei = rng.integers(0, cfg["N"], size=(2, cfg["E"])).astype(np.int64)
    W1 = (rng.normal(size=(cfg["DIN"], cfg["DH"])) / 16).astype(np.float32)
    b1 = (rng.normal(size=(cfg["DH"],)) * 0.1).astype(np.float32)
    W2 = (rng.normal(size=(cfg["DH"], cfg["DOUT"])) / 8).astype(np.float32)
    b2 = (rng.normal(size=(cfg["DOUT"],)) * 0.1).astype(np.float32)

    meta = preprocess(ei, cfg)
    print("c1:", meta["c1"], "slots:", meta["slots"],
          "pack_eff:", (cfg["E"] + cfg["N"]) / (meta["c1"] * CHUNK * CORES))
    got = emulate(x, W1, b1, W2, b2, meta, cfg)

    N = cfg["N"]
    loops = np.arange(N, dtype=np.int64)
    s = np.concatenate([ei[0], loops]); d = np.concatenate([ei[1], loops])
    deg = np.bincount(d, minlength=N).astype(np.float32)
    dis = np.where(deg > 0, 1 / np.sqrt(np.maximum(deg, 1)), 0).astype(np.float32)
    w = dis[s] * dis[d]

    def conv(xx, W, b):
        xw = xx @ W
        out = np.zeros((N, W.shape[1]), dtype=np.float32)
        np.add.at(out, d, xw[s] * w[:, None])
        return out + b

    h = np.maximum(conv(x, W1, b1), 0)
    o = conv(h, W2, b2)
    m = o.max(1, keepdims=True)
    ref = (o - m) - np.log(np.exp(o - m).sum(1, keepdims=True))
    err = np.abs(got - ref).max() / (np.abs(ref).max() + 1e-9)
    print("emulator vs ref max rel err:", err)
    assert err < 2e-3, err
    print("HOST LOGIC OK")


# revision 43
# speedup vs baseline: 1.4010x; 1.2975x over previous
"""2-layer GCN (gnn_message_passing) on 8 Trainium2 NeuronCores.

Strategy (graph/data parallel, dst-sharded, three SPMD launches):
  - Nodes sharded across 8 cores by destination id (12500 each). Host
    precomputes symmetric GCN normalization (graph preprocessing), adds
    self-loops, and bin-packs each core's nodes into uniform "chunks"
    (<=8 nodes, <=128 in-edges). Snake-balanced sharding (degree-sorted
    deal) plus a target-chasing packer with exact pair completion fill
    chunks to ~98% of the 128-lane capacity. One NEFF per stage runs
    SPMD on all 8 cores with per-core metadata tensors.
  - Per-edge random gather runs ON DEVICE via gpsimd indirect DMA
    (one offset per partition: each chunk's 128 lanes gather their
    source rows from a replicated fp8 feature table in one
    instruction). Each core therefore receives only the compact table
    (xw1: 6.4 MB, xw2: 4.3 MB) plus int32 per-lane row ids, instead of
    the ~2.4x-expanded per-edge feature streams.
  - The wall clock is dominated by host->device transfer (axon tunnel,
    ~36 MB/s serial), so every byte counts:
    * transform-first: launch 0 computes xw1 = x @ W1 on device, so
      the layer-1 table rows are 64 B fp8 instead of 512 B f32 raw x;
      the layer-2 table rows are 40 B fp8.
    * aggregation metadata per lane: int32 row id + fp16 norm weight
      (folded into the slot masks on device); per-chunk slot
      boundaries (9 fp16/chunk) expand to 0/1 masks via a DMA
      partition-broadcast + DVE is_le/sub, then scale by w.
    * end-to-end max rel err ~5e-3 (fp32 PSUM accumulation) vs the
      2e-2 gate.
  - Launch A: per-chunk gather + matmul msg^T @ mask -> feature-major
    PSUM, fused b1+ReLU (scalar engine), W2 matmul -> xw2 shard
    [slots, 40] fp8 (W2 matmul emits slot-major directly).
  - Host halo exchange: concatenates xw2 shards into the layer-2
    table; the device gathers it by source position in launch B.
  - Launch B: gather + aggregate the same way, PE-transpose to
    slot-major, add b2, log_softmax, write fp16 [slots, 40] per core.
  - Host un-permutes slot rows back to original node order.
"""

import numpy as np
import ml_dtypes

FULL = dict(N=100000, E=1600000, DIN=128, DH=64, DOUT=40)
CORES = 8
WSLOT = 8          # node slots per chunk
CHUNK = 128        # edge lanes per chunk
GRP = 16           # chunks per group  (GRP*WSLOT = 128 psum positions)
NP_F8 = ml_dtypes.float8_e4m3


# ------------------------------------------------------- host preprocessing
def _pack(degl):
    """Target-chasing bin-pack: <=WSLOT nodes, <=CHUNK edges per chunk.

    First item is the largest remaining degree; each further slot takes
    the available degree closest to cap/slots_left so chunks land near
    exactly CHUNK edges with ~WSLOT nodes (measured fill ~0.97).
    """
    n = len(degl)
    dmax = int(degl.max())
    by_deg = np.argsort(degl, kind="stable")
    startd = np.searchsorted(degl[by_deg], np.arange(dmax + 2))
    ptr = startd[1:].copy()              # pop position per degree bucket
    remaining = (startd[1:] - startd[:-1]).astype(np.int64)
    co = np.empty(n, np.int64)
    so = np.empty(n, np.int64)
    total, ci = n, 0
    while total > 0:
        # first: largest available
        d = dmax
        while d > 0 and remaining[d] == 0:
            d -= 1
        ptr[d] -= 1
        nl = by_deg[ptr[d]]
        remaining[d] -= 1
        total -= 1
        co[nl], so[nl] = ci, 0
        cap, k = CHUNK - d, 1
        while k < WSLOT and total > 0 and cap > 0:
            best, bestkey = 0, None
            if k == WSLOT - 2 and cap >= 2:
                # exact pair completion: pick d so that cap-d is also
                # available; the last slot then fills the chunk to CHUNK
                for dd in range(max(1, cap - dmax), min(cap - 1, dmax) + 1):
                    d2 = cap - dd
                    if d2 < 1 or d2 > dmax:
                        continue
                    if remaining[dd] >= (2 if d2 == dd else 1) \
                            and remaining[d2] >= 1:
                        key = (abs(dd - cap / 2), -dd)
                        if bestkey is None or key < bestkey:
                            bestkey, best = key, dd
            if best == 0:
                tgt = cap / (WSLOT - k)
                for d in range(1, min(cap, dmax) + 1):
                    if remaining[d] == 0:
                        continue
                    key = (abs(d - tgt), -d)
                    if bestkey is None or key < bestkey:
                        bestkey, best = key, d
            if best == 0:
                break
            ptr[best] -= 1
            nl = by_deg[ptr[best]]
            remaining[best] -= 1
            total -= 1
            co[nl], so[nl] = ci, k
            cap -= best
            k += 1
        ci += 1
    return co, so, ci


def preprocess(edge_index, cfg):
    """Graph preprocessing: norm weights, sharding, chunk packing.

    Returns per-core src ids / norm weight per edge lane
    ([CORES, CHUNK, c1]), per-chunk slot boundaries ([CORES, c1, 9]),
    slot maps, and the uniform chunk count c1.
    """
    N, NSH = cfg["N"], cfg["N"] // CORES
    src = np.asarray(edge_index[0], dtype=np.int64)
    dst = np.asarray(edge_index[1], dtype=np.int64)
    loops = np.arange(N, dtype=np.int64)
    s_all = np.concatenate([src, loops])
    d_all = np.concatenate([dst, loops])
    deg = np.bincount(d_all, minlength=N)
    dis = np.where(deg > 0, 1.0 / np.sqrt(np.maximum(deg, 1.0)), 0.0)
    dis = dis.astype(np.float32)

    o = np.argsort(d_all, kind="stable")
    s_srt, d_srt = s_all[o], d_all[o]
    w_srt = dis[s_srt] * dis[d_srt]
    seg = np.zeros(N + 1, np.int64)
    seg[1:] = np.cumsum(deg)

    # snake-balanced dst sharding: deal degree-sorted nodes across cores
    # so every core gets a near-identical degree multiset (equal packing)
    order = np.argsort(-deg, kind="stable")
    idx = np.arange(N)
    pos = idx % CORES
    core_rank = np.where((idx // CORES) % 2 == 0, pos, CORES - 1 - pos)
    nodes = [order[core_rank == c] for c in range(CORES)]

    packres = []
    nch = np.zeros(CORES, np.int64)
    for c in range(CORES):
        degl = deg[nodes[c]]
        assert degl.max() <= CHUNK, "node degree exceeds chunk capacity"
        assert degl.min() >= 1
        co, so, ncc = _pack(degl)
        packres.append((co, so))
        nch[c] = ncc

    c1 = ((int(nch.max()) + GRP - 1) // GRP) * GRP
    slots = c1 * WSLOT

    pos_of = np.empty(N, np.int64)
    rowpos = np.empty(N, np.int64)
    slot2node = np.full((CORES, slots), -1, np.int64)
    srcs = np.zeros((CORES, CHUNK, c1), np.int64)
    wml = np.zeros((CORES, CHUNK, c1), np.float16)
    bnd = np.zeros((CORES, c1, WSLOT + 1), np.float16)

    for c in range(CORES):
        ndc = nodes[c]
        co, so = packres[c]
        degl = deg[ndc]
        # lane base per node: exclusive cumsum of degrees in (chunk, slot) order
        ordk = np.argsort(co * WSLOT + so)
        degk = degl[ordk]
        cs = np.cumsum(degk) - degk
        cid = co[ordk]
        first = np.searchsorted(cid, np.arange(nch[c]), side="left")
        lane_base = np.empty(NSH, np.int64)
        lane_base[ordk] = cs - cs[first][cid]
        # ragged expansion of this core's edges (dst-sorted rows per node)
        lens = degl
        tot = int(lens.sum())
        cum = np.cumsum(lens) - lens
        within = np.arange(tot) - np.repeat(cum, lens)
        rows = np.repeat(seg[ndc], lens) + within
        eloc = np.repeat(np.arange(NSH), lens)
        lane_e = lane_base[eloc] + within
        assert lane_e.max() < CHUNK
        srcs[c, lane_e, co[eloc]] = s_srt[rows]
        wml[c, lane_e, co[eloc]] = w_srt[rows]
        pos_of[ndc] = c * slots + co * WSLOT + so
        rowpos[ndc] = c * NSH + np.arange(NSH)
        slot2node[c, co * WSLOT + so] = ndc
        # per-chunk slot boundaries: bnd[ci, s] = first lane of slot s,
        # bnd[ci, 8] = chunk fill; empty slots / pad chunks collapse to fill
        fill = np.zeros(c1, np.int64)
        np.add.at(fill, co, degl)
        bnd[c] = np.repeat(fill[:, None], WSLOT + 1, axis=1)
        bnd[c, co, so] = lane_base

    return dict(srcs=srcs, wml=wml, bnd=bnd, slot2node=slot2node,
                pos_of=pos_of, rowpos=rowpos, nodes=nodes, c1=c1, slots=slots)


# ------------------------------------------------------- numpy emulation
def emulate(x, W1, b1, W2, b2, meta, cfg):
    """Pure-numpy emulation of the device dataflow (logic validation)."""
    DOUT = cfg["DOUT"]
    c1, slots = meta["c1"], meta["slots"]
    srcs = meta["srcs"]
    lane = np.arange(CHUNK, dtype=np.float32)
    ge = meta["bnd"].astype(np.float32)[:, None, :, :] <= \
        lane[None, :, None, None]                 # [CORES, CHUNK, c1, 9]
    oh = ge[..., :WSLOT].astype(np.float32) - ge[..., 1:].astype(np.float32)
    wf = meta["wml"].astype(np.float32)[..., None]
    xw1 = x @ W1
    xw2_all = np.zeros((CORES * slots, DOUT), np.float32)
    for c in range(CORES):
        msg = wf[c] * xw1[srcs[c]]                # [CHUNK, c1, DH]
        hrawT = np.einsum("pcf,pcs->fcs", msg, oh[c]).reshape(-1, slots)
        hT = np.maximum(hrawT + b1[:, None], 0.0)
        xw2_all[c * slots:(c + 1) * slots] = (W2.T @ hT).T
    out_full = np.zeros((cfg["N"], DOUT), np.float32)
    for c in range(CORES):
        msg = wf[c] * xw2_all[meta["pos_of"][srcs[c]]]
        oT = np.einsum("pcf,pcs->fcs", msg, oh[c]).reshape(DOUT, slots)
        o = oT.T + b2[None, :]
        m = o.max(axis=1, keepdims=True)
        ls = (o - m) - np.log(np.exp(o - m).sum(axis=1, keepdims=True))
        sel = meta["slot2node"][c] >= 0
        out_full[meta["slot2node"][c][sel]] = ls[sel]
    return out_full


# ------------------------------------------------------- bass programs
def _bass_mods():
    import concourse.bass as bass
    import concourse.bacc as bacc
    import concourse.mybir as mybir
    import concourse.tile as tile
    return bass, bacc, mybir, tile


def _build_onehot(nc, tc, cpool, mybir, bnd_d, iota_d, w_d, c1):
    """Expand per-chunk slot boundaries to w-weighted masks [128, c1, 8].

    bnd[ci, s] is the first lane of slot s (bnd[ci, 8] = chunk fill);
    mask[lane, ci, s] = w[lane, ci] * (bnd[s] <= lane < bnd[s+1]). Pad
    lanes and pad chunks fall outside every [bnd[s], bnd[s+1]) interval
    and carry w = 0, so they are masked out structurally.
    """
    F16 = mybir.dt.float16
    F32 = mybir.dt.float32
    NB = WSLOT + 1
    ALU = mybir.AluOpType
    iota_s = cpool.tile([CHUNK, 1], F32)
    nc.sync.dma_start(iota_s[:], iota_d[:].unsqueeze(1))
    w_s = cpool.tile([CHUNK, c1], F16)
    nc.sync.dma_start(w_s[:], w_d[:, :])
    oh = cpool.tile([CHUNK, c1, WSLOT], F16)
    with tc.tile_pool(name="ohtmp", bufs=1) as tpool:
        bb = tpool.tile([CHUNK, c1, NB], F16)
        nc.sync.dma_start(
            bb[:], bnd_d[:, :].unsqueeze(0).broadcast_to([CHUNK, c1, NB]))
        ge = tpool.tile([CHUNK, c1, NB], F16)
        for s in range(NB):
            nc.vector.tensor_scalar(ge[:, :, s], bb[:, :, s], iota_s[:], None,
                                    ALU.is_le)
        mask = tpool.tile([CHUNK, c1], F16)
        for s in range(WSLOT):
            nc.vector.tensor_tensor(mask[:], ge[:, :, s], ge[:, :, s + 1],
                                    ALU.subtract)
            nc.vector.tensor_tensor(oh[:, :, s], mask[:], w_s[:], ALU.mult)
    return oh


def build_nc_0(cfg, nsh):
    """Launch 0: xw1 = x @ W1 per node shard (transform-first)."""
    bass, bacc, mybir, tile = _bass_mods()
    DIN, DH = cfg["DIN"], cfg["DH"]
    F8, F16, F32 = mybir.dt.float8e4, mybir.dt.float16, mybir.dt.float32
    PS = bass.MemorySpace.PSUM

    nc = bacc.Bacc(None, target_bir_lowering=False, num_devices=CORES)
    xT_d = nc.dram_tensor("xT", [DIN, nsh], F8, kind="ExternalInput")
    w1_d = nc.dram_tensor("W1", [DIN, DH], F16, kind="ExternalInput")
    xw1_d = nc.dram_tensor("xw1", [nsh, DH], F8, kind="ExternalOutput")

    with tile.TileContext(nc) as tc:
        with tc.tile_pool(name="const", bufs=1) as cpool:
            w1_s = cpool.tile([DIN, DH], F16)
            nc.sync.dma_start(w1_s[:], w1_d[:, :])
            xT_s = cpool.tile([DIN, nsh], F8)
            nc.sync.dma_start(xT_s[:], xT_d[:, :])
            with (
                tc.tile_pool(name="work", bufs=3) as wpool,
                tc.tile_pool(name="ps", bufs=4, space=PS) as pp,
            ):
                for t in range((nsh + 127) // 128):
                    n0 = t * 128
                    nn = min(128, nsh - n0)
                    p = pp.tile([128, DH], F32, tag="p")
                    nc.tensor.matmul(p[0:nn, :], xT_s[:, n0:n0 + nn],
                                     w1_s[:], start=True, stop=True)
                    ot = wpool.tile([128, DH], F8, tag="ot")
                    nc.vector.tensor_copy(ot[0:nn, :], p[0:nn, :])
                    nc.sync.dma_start(xw1_d[n0:n0 + nn, :], ot[0:nn, :])
    nc.compile()
    return nc


def build_nc_A(cfg, c1, ntab):
    """Launch A: gather xw1 rows, layer-1 aggregation + b1/relu/W2."""
    bass, bacc, mybir, tile = _bass_mods()
    DH, DOUT = cfg["DH"], cfg["DOUT"]
    F8, F16, F32 = mybir.dt.float8e4, mybir.dt.float16, mybir.dt.float32
    I32 = mybir.dt.int32
    AF = mybir.ActivationFunctionType
    PS = bass.MemorySpace.PSUM
    slots, ng = c1 * WSLOT, c1 // GRP

    nc = bacc.Bacc(None, target_bir_lowering=False, num_devices=CORES)
    tab_d = nc.dram_tensor("tab", [ntab, DH], F8, kind="ExternalInput")
    idx_d = nc.dram_tensor("idx", [CHUNK, c1], I32, kind="ExternalInput")
    w_d = nc.dram_tensor("w", [CHUNK, c1], F16, kind="ExternalInput")
    bnd_d = nc.dram_tensor("bnd", [c1, WSLOT + 1], F16, kind="ExternalInput")
    iota_d = nc.dram_tensor("iota", [CHUNK], F32, kind="ExternalInput")
    w2_d = nc.dram_tensor("W2", [DH, DOUT], F16, kind="ExternalInput")
    b1_d = nc.dram_tensor("b1", [DH], F32, kind="ExternalInput")
    xw2_d = nc.dram_tensor("xw2", [slots, DOUT], F8, kind="ExternalOutput")

    with tile.TileContext(nc) as tc:
        with tc.tile_pool(name="const", bufs=1) as cpool:
            w2_s = cpool.tile([DH, DOUT], F16)
            nc.sync.dma_start(w2_s[:], w2_d[:, :])
            b1_s = cpool.tile([DH, 1], F32)
            nc.sync.dma_start(b1_s[:], b1_d[:].unsqueeze(1))
            idx_s = cpool.tile([CHUNK, c1], I32)
            nc.sync.dma_start(idx_s[:], idx_d[:, :])
            oh = _build_onehot(nc, tc, cpool, mybir, bnd_d, iota_d, w_d, c1)
            with (
                tc.tile_pool(name="gath", bufs=2) as gpool,
                tc.tile_pool(name="work", bufs=2) as wpool,
                tc.tile_pool(name="ps1", bufs=2, space=PS) as pp,
                tc.tile_pool(name="ps2", bufs=2, space=PS) as ppb,
            ):
                for g in range(ng):
                    msg = gpool.tile([CHUNK, GRP, DH], F8, tag="msg")
                    for c in range(GRP):
                        nc.gpsimd.indirect_dma_start(
                            out=msg[:, c, :], out_offset=None, in_=tab_d[:],
                            in_offset=bass.IndirectOffsetOnAxis(
                                ap=idx_s[:, g * GRP + c:g * GRP + c + 1],
                                axis=0))
                    pg = pp.tile([DH, GRP * WSLOT], F32, tag="agg")
                    for c in range(GRP):
                        nc.tensor.matmul(
                            pg[:, c * WSLOT:(c + 1) * WSLOT],
                            msg[:, c, :], oh[:, g * GRP + c, :],
                            start=True, stop=True)
                    hT = wpool.tile([DH, GRP * WSLOT], F16, tag="hT")
                    nc.scalar.activation(hT[:], pg[:], AF.Relu, bias=b1_s[:])
                    for k in range(GRP * WSLOT // 128):
                        p2 = ppb.tile([128, DOUT], F32, tag="p2")
                        nc.tensor.matmul(p2[:], hT[:, k * 128:(k + 1) * 128],
                                         w2_s[:], start=True, stop=True)
                        ot = wpool.tile([128, DOUT], F8, tag="ot")
                        nc.vector.tensor_copy(ot[:], p2[:])
                        r0 = (g * (GRP * WSLOT // 128) + k) * 128
                        nc.sync.dma_start(xw2_d[r0:r0 + 128, :], ot[:])
    nc.compile()
    return nc


def build_nc_B(cfg, c1, ntab):
    """Launch B: gather xw2 rows, layer-2 aggregation + b2 + log_softmax."""
    bass, bacc, mybir, tile = _bass_mods()
    DOUT = cfg["DOUT"]
    F8, F16, F32 = mybir.dt.float8e4, mybir.dt.float16, mybir.dt.float32
    I32 = mybir.dt.int32
    AF = mybir.ActivationFunctionType
    ALU = mybir.AluOpType
    AX = mybir.AxisListType
    PS = bass.MemorySpace.PSUM
    slots, ng = c1 * WSLOT, c1 // GRP

    nc = bacc.Bacc(None, target_bir_lowering=False, num_devices=CORES)
    tab_d = nc.dram_tensor("tab", [ntab, DOUT], F8, kind="ExternalInput")
    idx_d = nc.dram_tensor("idx", [CHUNK, c1], I32, kind="ExternalInput")
    w_d = nc.dram_tensor("w", [CHUNK, c1], F16, kind="ExternalInput")
    bnd_d = nc.dram_tensor("bnd", [c1, WSLOT + 1], F16, kind="ExternalInput")
    iota_d = nc.dram_tensor("iota", [CHUNK], F32, kind="ExternalInput")
    b2_d = nc.dram_tensor("b2", [DOUT], F32, kind="ExternalInput")
    id_d = nc.dram_tensor("ident", [DOUT, DOUT], F32, kind="ExternalInput")
    out_d = nc.dram_tensor("out", [slots, DOUT], F16, kind="ExternalOutput")

    with tile.TileContext(nc) as tc:
        with tc.tile_pool(name="const", bufs=1) as cpool:
            id_s = cpool.tile([DOUT, DOUT], F32)
            nc.sync.dma_start(id_s[:], id_d[:, :])
            b2r_s = cpool.tile([1, DOUT], F32)
            nc.sync.dma_start(b2r_s[:], b2_d[:].unsqueeze(0))
            ones_s = cpool.tile([1, 128], F32)
            nc.vector.memset(ones_s[:], 1.0)
            b2b_s = cpool.tile([128, DOUT], F32)
            with tc.tile_pool(name="pbc", bufs=1, space=PS) as pbc:
                pb = pbc.tile([128, DOUT], F32)
                nc.tensor.matmul(pb[:], ones_s[:], b2r_s[:], start=True, stop=True)
                nc.vector.tensor_copy(b2b_s[:], pb[:])
            idx_s = cpool.tile([CHUNK, c1], I32)
            nc.sync.dma_start(idx_s[:], idx_d[:, :])
            oh = _build_onehot(nc, tc, cpool, mybir, bnd_d, iota_d, w_d, c1)
            with (
                tc.tile_pool(name="gath", bufs=2) as gpool,
                tc.tile_pool(name="work", bufs=2) as wpool,
                tc.tile_pool(name="ps1", bufs=2, space=PS) as pp,
                tc.tile_pool(name="ps2", bufs=2, space=PS) as ppb,
            ):
                for g in range(ng):
                    msg = gpool.tile([CHUNK, GRP, DOUT], F8, tag="msg")
                    for c in range(GRP):
                        nc.gpsimd.indirect_dma_start(
                            out=msg[:, c, :], out_offset=None, in_=tab_d[:],
                            in_offset=bass.IndirectOffsetOnAxis(
                                ap=idx_s[:, g * GRP + c:g * GRP + c + 1],
                                axis=0))
                    pg = pp.tile([DOUT, GRP * WSLOT], F32, tag="agg")
                    for c in range(GRP):
                        nc.tensor.matmul(
                            pg[:, c * WSLOT:(c + 1) * WSLOT],
                            msg[:, c, :], oh[:, g * GRP + c, :],
                            start=True, stop=True)
                    oT = wpool.tile([DOUT, GRP * WSLOT], F32, tag="oT")
                    nc.scalar.copy(oT[:], pg[:])
                    for k in range(GRP * WSLOT // 128):
                        pt = ppb.tile([128, DOUT], F32, tag="pt")
                        nc.tensor.transpose(pt[:], oT[:, k * 128:(k + 1) * 128],
                                            id_s[:])
                        t = wpool.tile([128, DOUT], F32, tag="t")
                        nc.vector.tensor_tensor(t[:], pt[:], b2b_s[:], ALU.add)
                        mx = wpool.tile([128, 1], F32, tag="mx")
                        nc.vector.tensor_reduce(mx[:], t[:], AX.X, ALU.max)
                        sh = wpool.tile([128, DOUT], F32, tag="sh")
                        nc.vector.tensor_scalar_sub(sh[:], t[:], mx[:])
                        ex = wpool.tile([128, DOUT], F32, tag="ex")
                        nc.scalar.activation(ex[:], sh[:], AF.Exp)
                        sm = wpool.tile([128, 1], F32, tag="sm")
                        nc.vector.tensor_reduce(sm[:], ex[:], AX.X, ALU.add)
                        lg = wpool.tile([128, 1], F32, tag="lg")
                        nc.scalar.activation(lg[:], sm[:], AF.Ln)
                        res = wpool.tile([128, DOUT], F16, tag="res")
                        nc.vector.tensor_scalar_sub(res[:], sh[:], lg[:])
                        r0 = (g * (GRP * WSLOT // 128) + k) * 128
                        nc.sync.dma_start(out_d[r0:r0 + 128, :], res[:])
    nc.compile()
    return nc


# ------------------------------------------------------- public entry
def kernel(x, edge_index, W1, b1, W2, b2, cfg=None, trace=False, time_reps=0):
    import time as _time

    from concourse.bass_utils import run_bass_kernel_spmd

    cfg = cfg or FULL
    N, NSH = cfg["N"], cfg["N"] // CORES
    DIN, DH, DOUT = cfg["DIN"], cfg["DH"], cfg["DOUT"]
    x = np.ascontiguousarray(np.asarray(x, dtype=np.float32))
    W1_h = np.asarray(W1, dtype=np.float32).astype(np.float16)
    b1_h = np.asarray(b1, dtype=np.float32)
    W2_h = np.asarray(W2, dtype=np.float32).astype(np.float16)
    b2_h = np.asarray(b2, dtype=np.float32)
    ident = np.eye(DOUT, dtype=np.float32)

    meta = preprocess(edge_index, cfg)
    c1, slots = meta["c1"], meta["slots"]

    def timed(nc, ins, store):
        res = run_bass_kernel_spmd(nc, ins, core_ids=list(range(CORES)),
                                   trace=trace)
        for _ in range(time_reps):
            t0 = _time.perf_counter()
            run_bass_kernel_spmd(nc, ins, core_ids=list(range(CORES)))
            store.append(_time.perf_counter() - t0)
        return res

    # ---- launch 0: xw1 = x @ W1 ----
    xq = x.astype(NP_F8)
    xT_in = [np.ascontiguousarray(xq[meta["nodes"][c]].T)
             for c in range(CORES)]
    nc_0 = build_nc_0(cfg, NSH)
    in_0 = [{"xT": xT_in[c], "W1": W1_h} for c in range(CORES)]
    kernel.times_0 = []
    res_0 = timed(nc_0, in_0, kernel.times_0)
    xw1_all = np.concatenate(
        [res_0.results[c]["xw1"] for c in range(CORES)], 0)

    # ---- launch A: layer 1 (device gathers xw1 rows per edge) ----
    ref1 = meta["rowpos"][meta["srcs"]].astype(np.int32)
    lane_iota = np.arange(CHUNK, dtype=np.float32)
    nc_a = build_nc_A(cfg, c1, N)
    in_a = [{"tab": xw1_all, "idx": ref1[c], "w": meta["wml"][c],
             "bnd": meta["bnd"][c], "iota": lane_iota,
             "W2": W2_h, "b1": b1_h} for c in range(CORES)]
    kernel.times_a = []
    res_a = timed(nc_a, in_a, kernel.times_a)

    # ---- host halo exchange ----
    xw2_all = np.concatenate(
        [res_a.results[c]["xw2"] for c in range(CORES)], 0)
    ref2 = meta["pos_of"][meta["srcs"]].astype(np.int32)

    # ---- launch B: layer 2 (device gathers xw2 rows per edge) ----
    nc_b = build_nc_B(cfg, c1, CORES * slots)
    in_b = [{"tab": xw2_all, "idx": ref2[c], "w": meta["wml"][c],
             "bnd": meta["bnd"][c], "iota": lane_iota,
             "b2": b2_h, "ident": ident} for c in range(CORES)]
    kernel.times_b = []
    res_b = timed(nc_b, in_b, kernel.times_b)

    out_full = np.zeros((N, DOUT), np.float32)
    for c in range(CORES):
        o = res_b.results[c]["out"].astype(np.float32)
        sel = meta["slot2node"][c] >= 0
        out_full[meta["slot2node"][c][sel]] = o[sel]
    return out_full


if __name__ == "__main__":
    cfg = dict(N=4096, E=65536, DIN=128, DH=64, DOUT=40)
    rng = np.random.default_rng(0)
    x = rng.normal(size=(cfg["N"], cfg["DIN"])).astype(np.float32)
    ei = rng.integers(0, cfg["N"], size=(2, cfg["E"])).astype(np.int64)
    W1 = (rng.normal(size=(cfg["DIN"], cfg["DH"])) / 16).astype(np.float32)
    b1 = (rng.normal(size=(cfg["DH"],)) * 0.1).astype(np.float32)
    W2 = (rng.normal(size=(cfg["DH"], cfg["DOUT"])) / 8).astype(np.float32)
    b2 = (rng.normal(size=(cfg["DOUT"],)) * 0.1).astype(np.float32)

    meta = preprocess(ei, cfg)
    print("c1:", meta["c1"], "slots:", meta["slots"],
          "pack_eff:", (cfg["E"] + cfg["N"]) / (meta["c1"] * CHUNK * CORES))
    got = emulate(x, W1, b1, W2, b2, meta, cfg)

    N = cfg["N"]
    loops = np.arange(N, dtype=np.int64)
    s = np.concatenate([ei[0], loops]); d = np.concatenate([ei[1], loops])
    deg = np.bincount(d, minlength=N).astype(np.float32)
    dis = np.where(deg > 0, 1 / np.sqrt(np.maximum(deg, 1)), 0).astype(np.float32)
    w = dis[s] * dis[d]

    def conv(xx, W, b):
        xw = xx @ W
        out = np.zeros((N, W.shape[1]), dtype=np.float32)
        np.add.at(out, d, xw[s] * w[:, None])
        return out + b

    h = np.maximum(conv(x, W1, b1), 0)
    o = conv(h, W2, b2)
    m = o.max(1, keepdims=True)
    ref = (o - m) - np.log(np.exp(o - m).sum(1, keepdims=True))
    err = np.abs(got - ref).max() / (np.abs(ref).max() + 1e-9)
    print("emulator vs ref max rel err:", err)
    assert err < 2e-3, err
    print("HOST LOGIC OK")


# revision 44
# speedup vs baseline: 1.4455x; 1.0318x over previous
"""2-layer GCN (gnn_message_passing) on 8 Trainium2 NeuronCores.

Strategy (graph/data parallel, dst-sharded, three SPMD launches):
  - Nodes sharded across 8 cores by destination id (12500 each). Host
    precomputes symmetric GCN normalization (graph preprocessing), adds
    self-loops, and bin-packs each core's nodes into uniform "chunks"
    (<=8 nodes, <=128 in-edges). Snake-balanced sharding (degree-sorted
    deal) plus a target-chasing packer with exact pair completion fill
    chunks to ~98% of the 128-lane capacity. One NEFF per stage runs
    SPMD on all 8 cores with per-core metadata tensors.
  - Per-edge random gather runs ON DEVICE via gpsimd indirect DMA
    (one offset per partition: each chunk's 128 lanes gather their
    source rows from a replicated fp8 feature table in one
    instruction). Each core therefore receives only the compact table
    (xw1: 6.4 MB, xw2: 4.3 MB) plus int32 per-lane row ids, instead of
    the ~2.4x-expanded per-edge feature streams.
  - The wall clock is dominated by host->device transfer (axon tunnel,
    ~36 MB/s serial), so every byte counts:
    * transform-first: launch 0 computes xw1 = x @ W1 on device, so
      the layer-1 table rows are 64 B fp8 instead of 512 B f32 raw x;
      the layer-2 table rows are 40 B fp8.
    * aggregation metadata per lane: int32 row id + fp16 norm weight
      (folded into the slot masks on device); per-chunk slot
      boundaries (9 fp16/chunk) expand to 0/1 masks via a DMA
      partition-broadcast + DVE is_le/sub, then scale by w.
    * end-to-end max rel err ~5e-3 (fp32 PSUM accumulation) vs the
      2e-2 gate.
  - Launch A: per-chunk gather + matmul msg^T @ mask -> feature-major
    PSUM, fused b1+ReLU (scalar engine), W2 matmul -> xw2 shard
    [slots, 40] fp8 (W2 matmul emits slot-major directly).
  - Host halo exchange: concatenates xw2 shards into the layer-2
    table; the device gathers it by source position in launch B.
  - Launch B: gather + aggregate the same way, PE-transpose to
    slot-major, add b2, log_softmax, write fp16 [slots, 40] per core.
  - Host un-permutes slot rows back to original node order.
"""

import numpy as np
import ml_dtypes

FULL = dict(N=100000, E=1600000, DIN=128, DH=64, DOUT=40)
CORES = 8
WSLOT = 8          # node slots per chunk
CHUNK = 128        # edge lanes per chunk
GRP = 16           # chunks per group  (GRP*WSLOT = 128 psum positions)
NP_F8 = ml_dtypes.float8_e4m3


# ------------------------------------------------------- host preprocessing
def _pack(degl):
    """Target-chasing bin-pack: <=WSLOT nodes, <=CHUNK edges per chunk.

    First item is the largest remaining degree; each further slot takes
    the available degree closest to cap/slots_left so chunks land near
    exactly CHUNK edges with ~WSLOT nodes (measured fill ~0.97).
    """
    n = len(degl)
    dmax = int(degl.max())
    by_deg = np.argsort(degl, kind="stable")
    startd = np.searchsorted(degl[by_deg], np.arange(dmax + 2))
    ptr = startd[1:].copy()              # pop position per degree bucket
    remaining = (startd[1:] - startd[:-1]).astype(np.int64)
    co = np.empty(n, np.int64)
    so = np.empty(n, np.int64)
    total, ci = n, 0
    while total > 0:
        # first: largest available
        d = dmax
        while d > 0 and remaining[d] == 0:
            d -= 1
        ptr[d] -= 1
        nl = by_deg[ptr[d]]
        remaining[d] -= 1
        total -= 1
        co[nl], so[nl] = ci, 0
        cap, k = CHUNK - d, 1
        while k < WSLOT and total > 0 and cap > 0:
            best, bestkey = 0, None
            if k == WSLOT - 2 and cap >= 2:
                # exact pair completion: pick d so that cap-d is also
                # available; the last slot then fills the chunk to CHUNK
                for dd in range(max(1, cap - dmax), min(cap - 1, dmax) + 1):
                    d2 = cap - dd
                    if d2 < 1 or d2 > dmax:
                        continue
                    if remaining[dd] >= (2 if d2 == dd else 1) \
                            and remaining[d2] >= 1:
                        key = (abs(dd - cap / 2), -dd)
                        if bestkey is None or key < bestkey:
                            bestkey, best = key, dd
            if best == 0:
                tgt = cap / (WSLOT - k)
                for d in range(1, min(cap, dmax) + 1):
                    if remaining[d] == 0:
                        continue
                    key = (abs(d - tgt), -d)
                    if bestkey is None or key < bestkey:
                        bestkey, best = key, d
            if best == 0:
                break
            ptr[best] -= 1
            nl = by_deg[ptr[best]]
            remaining[best] -= 1
            total -= 1
            co[nl], so[nl] = ci, k
            cap -= best
            k += 1
        ci += 1
    return co, so, ci


def preprocess(edge_index, cfg):
    """Graph preprocessing: norm weights, sharding, chunk packing.

    Returns per-core src ids / norm weight per edge lane
    ([CORES, CHUNK, c1]), per-chunk slot boundaries ([CORES, c1, 9]),
    slot maps, and the uniform chunk count c1.
    """
    N, NSH = cfg["N"], cfg["N"] // CORES
    src = np.asarray(edge_index[0], dtype=np.int64)
    dst = np.asarray(edge_index[1], dtype=np.int64)
    loops = np.arange(N, dtype=np.int64)
    s_all = np.concatenate([src, loops])
    d_all = np.concatenate([dst, loops])
    deg = np.bincount(d_all, minlength=N)
    dis = np.where(deg > 0, 1.0 / np.sqrt(np.maximum(deg, 1.0)), 0.0)
    dis = dis.astype(np.float32)

    o = np.argsort(d_all, kind="stable")
    s_srt, d_srt = s_all[o], d_all[o]
    w_srt = dis[s_srt] * dis[d_srt]
    seg = np.zeros(N + 1, np.int64)
    seg[1:] = np.cumsum(deg)

    # snake-balanced dst sharding: deal degree-sorted nodes across cores
    # so every core gets a near-identical degree multiset (equal packing)
    order = np.argsort(-deg, kind="stable")
    idx = np.arange(N)
    pos = idx % CORES
    core_rank = np.where((idx // CORES) % 2 == 0, pos, CORES - 1 - pos)
    nodes = [order[core_rank == c] for c in range(CORES)]

    packres = []
    nch = np.zeros(CORES, np.int64)
    for c in range(CORES):
        degl = deg[nodes[c]]
        assert degl.max() <= CHUNK, "node degree exceeds chunk capacity"
        assert degl.min() >= 1
        co, so, ncc = _pack(degl)
        packres.append((co, so))
        nch[c] = ncc

    c1 = ((int(nch.max()) + GRP - 1) // GRP) * GRP
    slots = c1 * WSLOT

    pos_of = np.empty(N, np.int64)
    rowpos = np.empty(N, np.int64)
    slot2node = np.full((CORES, slots), -1, np.int64)
    srcs = np.zeros((CORES, CHUNK, c1), np.int64)
    wml = np.zeros((CORES, CHUNK, c1), np.float16)
    bnd = np.zeros((CORES, c1, WSLOT + 1), np.float16)

    for c in range(CORES):
        ndc = nodes[c]
        co, so = packres[c]
        degl = deg[ndc]
        # lane base per node: exclusive cumsum of degrees in (chunk, slot) order
        ordk = np.argsort(co * WSLOT + so)
        degk = degl[ordk]
        cs = np.cumsum(degk) - degk
        cid = co[ordk]
        first = np.searchsorted(cid, np.arange(nch[c]), side="left")
        lane_base = np.empty(NSH, np.int64)
        lane_base[ordk] = cs - cs[first][cid]
        # ragged expansion of this core's edges (dst-sorted rows per node)
        lens = degl
        tot = int(lens.sum())
        cum = np.cumsum(lens) - lens
        within = np.arange(tot) - np.repeat(cum, lens)
        rows = np.repeat(seg[ndc], lens) + within
        eloc = np.repeat(np.arange(NSH), lens)
        lane_e = lane_base[eloc] + within
        assert lane_e.max() < CHUNK
        srcs[c, lane_e, co[eloc]] = s_srt[rows]
        wml[c, lane_e, co[eloc]] = w_srt[rows]
        pos_of[ndc] = c * slots + co * WSLOT + so
        rowpos[ndc] = c * NSH + np.arange(NSH)
        slot2node[c, co * WSLOT + so] = ndc
        # per-chunk slot boundaries: bnd[ci, s] = first lane of slot s,
        # bnd[ci, 8] = chunk fill; empty slots / pad chunks collapse to fill
        fill = np.zeros(c1, np.int64)
        np.add.at(fill, co, degl)
        bnd[c] = np.repeat(fill[:, None], WSLOT + 1, axis=1)
        bnd[c, co, so] = lane_base

    return dict(srcs=srcs, wml=wml, bnd=bnd, slot2node=slot2node,
                pos_of=pos_of, rowpos=rowpos, nodes=nodes, c1=c1, slots=slots)


# ------------------------------------------------------- numpy emulation
def emulate(x, W1, b1, W2, b2, meta, cfg):
    """Pure-numpy emulation of the device dataflow (logic validation)."""
    DOUT = cfg["DOUT"]
    c1, slots = meta["c1"], meta["slots"]
    srcs = meta["srcs"]
    lane = np.arange(CHUNK, dtype=np.float32)
    ge = meta["bnd"].astype(np.float32)[:, None, :, :] <= \
        lane[None, :, None, None]                 # [CORES, CHUNK, c1, 9]
    oh = ge[..., :WSLOT].astype(np.float32) - ge[..., 1:].astype(np.float32)
    wf = meta["wml"].astype(np.float32)[..., None]
    xw1 = x @ W1
    xw2_all = np.zeros((CORES * slots, DOUT), np.float32)
    for c in range(CORES):
        msg = wf[c] * xw1[srcs[c]]                # [CHUNK, c1, DH]
        hrawT = np.einsum("pcf,pcs->fcs", msg, oh[c]).reshape(-1, slots)
        hT = np.maximum(hrawT + b1[:, None], 0.0)
        xw2_all[c * slots:(c + 1) * slots] = (W2.T @ hT).T
    out_full = np.zeros((cfg["N"], DOUT), np.float32)
    for c in range(CORES):
        msg = wf[c] * xw2_all[meta["pos_of"][srcs[c]]]
        oT = np.einsum("pcf,pcs->fcs", msg, oh[c]).reshape(DOUT, slots)
        o = oT.T + b2[None, :]
        m = o.max(axis=1, keepdims=True)
        ls = (o - m) - np.log(np.exp(o - m).sum(axis=1, keepdims=True))
        sel = meta["slot2node"][c] >= 0
        out_full[meta["slot2node"][c][sel]] = ls[sel]
    return out_full


# ------------------------------------------------------- bass programs
def _bass_mods():
    import concourse.bass as bass
    import concourse.bacc as bacc
    import concourse.mybir as mybir
    import concourse.tile as tile
    return bass, bacc, mybir, tile


def _build_onehot(nc, tc, cpool, mybir, bnd_d, iota_d, w_d, c1):
    """Expand per-chunk slot boundaries to w-weighted masks [128, c1, 8].

    bnd[ci, s] is the first lane of slot s (bnd[ci, 8] = chunk fill);
    mask[lane, ci, s] = w[lane, ci] * (bnd[s] <= lane < bnd[s+1]). Pad
    lanes and pad chunks fall outside every [bnd[s], bnd[s+1]) interval
    and carry w = 0, so they are masked out structurally.
    """
    F16 = mybir.dt.float16
    F32 = mybir.dt.float32
    NB = WSLOT + 1
    ALU = mybir.AluOpType
    iota_s = cpool.tile([CHUNK, 1], F32)
    nc.sync.dma_start(iota_s[:], iota_d[:].unsqueeze(1))
    w_s = cpool.tile([CHUNK, c1], F16)
    nc.sync.dma_start(w_s[:], w_d[:, :])
    oh = cpool.tile([CHUNK, c1, WSLOT], F16)
    with tc.tile_pool(name="ohtmp", bufs=1) as tpool:
        bb = tpool.tile([CHUNK, c1, NB], F16)
        nc.sync.dma_start(
            bb[:], bnd_d[:, :].unsqueeze(0).broadcast_to([CHUNK, c1, NB]))
        ge = tpool.tile([CHUNK, c1, NB], F16)
        for s in range(NB):
            nc.vector.tensor_scalar(ge[:, :, s], bb[:, :, s], iota_s[:], None,
                                    ALU.is_le)
        mask = tpool.tile([CHUNK, c1], F16)
        for s in range(WSLOT):
            nc.vector.tensor_tensor(mask[:], ge[:, :, s], ge[:, :, s + 1],
                                    ALU.subtract)
            nc.vector.tensor_tensor(oh[:, :, s], mask[:], w_s[:], ALU.mult)
    return oh


def build_nc_0(cfg, nsh):
    """Launch 0: xw1 = x @ W1 per node shard (transform-first)."""
    bass, bacc, mybir, tile = _bass_mods()
    DIN, DH = cfg["DIN"], cfg["DH"]
    F8, F16, F32 = mybir.dt.float8e4, mybir.dt.float16, mybir.dt.float32
    PS = bass.MemorySpace.PSUM

    nc = bacc.Bacc(None, target_bir_lowering=False, num_devices=CORES)
    xT_d = nc.dram_tensor("xT", [DIN, nsh], F8, kind="ExternalInput")
    w1_d = nc.dram_tensor("W1", [DIN, DH], F16, kind="ExternalInput")
    xw1_d = nc.dram_tensor("xw1", [nsh, DH], F8, kind="ExternalOutput")

    with tile.TileContext(nc) as tc:
        with tc.tile_pool(name="const", bufs=1) as cpool:
            w1_s = cpool.tile([DIN, DH], F16)
            nc.sync.dma_start(w1_s[:], w1_d[:, :])
            xT_s = cpool.tile([DIN, nsh], F8)
            nc.sync.dma_start(xT_s[:], xT_d[:, :])
            with (
                tc.tile_pool(name="work", bufs=3) as wpool,
                tc.tile_pool(name="ps", bufs=4, space=PS) as pp,
            ):
                for t in range((nsh + 127) // 128):
                    n0 = t * 128
                    nn = min(128, nsh - n0)
                    p = pp.tile([128, DH], F32, tag="p")
                    nc.tensor.matmul(p[0:nn, :], xT_s[:, n0:n0 + nn],
                                     w1_s[:], start=True, stop=True)
                    ot = wpool.tile([128, DH], F8, tag="ot")
                    nc.vector.tensor_copy(ot[0:nn, :], p[0:nn, :])
                    nc.sync.dma_start(xw1_d[n0:n0 + nn, :], ot[0:nn, :])
    nc.compile()
    return nc


def build_nc_A(cfg, c1, ntab):
    """Launch A: gather xw1 rows, layer-1 aggregation + b1/relu/W2."""
    bass, bacc, mybir, tile = _bass_mods()
    DH, DOUT = cfg["DH"], cfg["DOUT"]
    F8, F16, F32 = mybir.dt.float8e4, mybir.dt.float16, mybir.dt.float32
    I32 = mybir.dt.int32
    AF = mybir.ActivationFunctionType
    PS = bass.MemorySpace.PSUM
    slots, ng = c1 * WSLOT, c1 // GRP

    nc = bacc.Bacc(None, target_bir_lowering=False, num_devices=CORES)
    tab_d = nc.dram_tensor("tab", [ntab, DH], F8, kind="ExternalInput")
    idx_d = nc.dram_tensor("idx", [CHUNK, c1], I32, kind="ExternalInput")
    w_d = nc.dram_tensor("w", [CHUNK, c1], F16, kind="ExternalInput")
    bnd_d = nc.dram_tensor("bnd", [c1, WSLOT + 1], F16, kind="ExternalInput")
    iota_d = nc.dram_tensor("iota", [CHUNK], F32, kind="ExternalInput")
    w2_d = nc.dram_tensor("W2", [DH, DOUT], F16, kind="ExternalInput")
    b1_d = nc.dram_tensor("b1", [DH], F32, kind="ExternalInput")
    xw2_d = nc.dram_tensor("xw2", [slots, DOUT], F8, kind="ExternalOutput")

    with tile.TileContext(nc) as tc:
        with tc.tile_pool(name="const", bufs=1) as cpool:
            w2_s = cpool.tile([DH, DOUT], F16)
            nc.sync.dma_start(w2_s[:], w2_d[:, :])
            b1_s = cpool.tile([DH, 1], F32)
            nc.sync.dma_start(b1_s[:], b1_d[:].unsqueeze(1))
            idx_s = cpool.tile([CHUNK, c1], I32)
            nc.sync.dma_start(idx_s[:], idx_d[:, :])
            oh = _build_onehot(nc, tc, cpool, mybir, bnd_d, iota_d, w_d, c1)
            with (
                tc.tile_pool(name="gath", bufs=2) as gpool,
                tc.tile_pool(name="work", bufs=2) as wpool,
                tc.tile_pool(name="ps1", bufs=2, space=PS) as pp,
                tc.tile_pool(name="ps2", bufs=2, space=PS) as ppb,
            ):
                for g in range(ng):
                    msg = gpool.tile([CHUNK, GRP, DH], F8, tag="msg")
                    for c in range(GRP):
                        nc.gpsimd.indirect_dma_start(
                            out=msg[:, c, :], out_offset=None, in_=tab_d[:],
                            in_offset=bass.IndirectOffsetOnAxis(
                                ap=idx_s[:, g * GRP + c:g * GRP + c + 1],
                                axis=0))
                    pg = pp.tile([DH, GRP * WSLOT], F32, tag="agg")
                    for c in range(GRP):
                        nc.tensor.matmul(
                            pg[:, c * WSLOT:(c + 1) * WSLOT],
                            msg[:, c, :], oh[:, g * GRP + c, :],
                            start=True, stop=True)
                    hT = wpool.tile([DH, GRP * WSLOT], F16, tag="hT")
                    nc.scalar.activation(hT[:], pg[:], AF.Relu, bias=b1_s[:])
                    for k in range(GRP * WSLOT // 128):
                        p2 = ppb.tile([128, DOUT], F32, tag="p2")
                        nc.tensor.matmul(p2[:], hT[:, k * 128:(k + 1) * 128],
                                         w2_s[:], start=True, stop=True)
                        ot = wpool.tile([128, DOUT], F8, tag="ot")
                        nc.vector.tensor_copy(ot[:], p2[:])
                        r0 = (g * (GRP * WSLOT // 128) + k) * 128
                        nc.sync.dma_start(xw2_d[r0:r0 + 128, :], ot[:])
    nc.compile()
    return nc


def build_nc_B(cfg, c1, ntab):
    """Launch B: gather xw2 rows, layer-2 aggregation + b2 + log_softmax."""
    bass, bacc, mybir, tile = _bass_mods()
    DOUT = cfg["DOUT"]
    F8, F16, F32 = mybir.dt.float8e4, mybir.dt.float16, mybir.dt.float32
    I32 = mybir.dt.int32
    AF = mybir.ActivationFunctionType
    ALU = mybir.AluOpType
    AX = mybir.AxisListType
    PS = bass.MemorySpace.PSUM
    slots, ng = c1 * WSLOT, c1 // GRP

    nc = bacc.Bacc(None, target_bir_lowering=False, num_devices=CORES)
    tab_d = nc.dram_tensor("tab", [ntab, DOUT], F8, kind="ExternalInput")
    idx_d = nc.dram_tensor("idx", [CHUNK, c1], I32, kind="ExternalInput")
    w_d = nc.dram_tensor("w", [CHUNK, c1], F16, kind="ExternalInput")
    bnd_d = nc.dram_tensor("bnd", [c1, WSLOT + 1], F16, kind="ExternalInput")
    iota_d = nc.dram_tensor("iota", [CHUNK], F32, kind="ExternalInput")
    b2_d = nc.dram_tensor("b2", [DOUT], F32, kind="ExternalInput")
    id_d = nc.dram_tensor("ident", [DOUT, DOUT], F32, kind="ExternalInput")
    out_d = nc.dram_tensor("out", [slots, DOUT], F16, kind="ExternalOutput")

    with tile.TileContext(nc) as tc:
        with tc.tile_pool(name="const", bufs=1) as cpool:
            id_s = cpool.tile([DOUT, DOUT], F32)
            nc.sync.dma_start(id_s[:], id_d[:, :])
            b2r_s = cpool.tile([1, DOUT], F32)
            nc.sync.dma_start(b2r_s[:], b2_d[:].unsqueeze(0))
            ones_s = cpool.tile([1, 128], F32)
            nc.vector.memset(ones_s[:], 1.0)
            b2b_s = cpool.tile([128, DOUT], F32)
            with tc.tile_pool(name="pbc", bufs=1, space=PS) as pbc:
                pb = pbc.tile([128, DOUT], F32)
                nc.tensor.matmul(pb[:], ones_s[:], b2r_s[:], start=True, stop=True)
                nc.vector.tensor_copy(b2b_s[:], pb[:])
            idx_s = cpool.tile([CHUNK, c1], I32)
            nc.sync.dma_start(idx_s[:], idx_d[:, :])
            oh = _build_onehot(nc, tc, cpool, mybir, bnd_d, iota_d, w_d, c1)
            with (
                tc.tile_pool(name="gath", bufs=2) as gpool,
                tc.tile_pool(name="work", bufs=2) as wpool,
                tc.tile_pool(name="ps1", bufs=2, space=PS) as pp,
                tc.tile_pool(name="ps2", bufs=2, space=PS) as ppb,
            ):
                for g in range(ng):
                    msg = gpool.tile([CHUNK, GRP, DOUT], F8, tag="msg")
                    for c in range(GRP):
                        nc.gpsimd.indirect_dma_start(
                            out=msg[:, c, :], out_offset=None, in_=tab_d[:],
                            in_offset=bass.IndirectOffsetOnAxis(
                                ap=idx_s[:, g * GRP + c:g * GRP + c + 1],
                                axis=0))
                    pg = pp.tile([DOUT, GRP * WSLOT], F32, tag="agg")
                    for c in range(GRP):
                        nc.tensor.matmul(
                            pg[:, c * WSLOT:(c + 1) * WSLOT],
                            msg[:, c, :], oh[:, g * GRP + c, :],
                            start=True, stop=True)
                    oT = wpool.tile([DOUT, GRP * WSLOT], F32, tag="oT")
                    nc.scalar.copy(oT[:], pg[:])
                    for k in range(GRP * WSLOT // 128):
                        pt = ppb.tile([128, DOUT], F32, tag="pt")
                        nc.tensor.transpose(pt[:], oT[:, k * 128:(k + 1) * 128],
                                            id_s[:])
                        t = wpool.tile([128, DOUT], F32, tag="t")
                        nc.vector.tensor_tensor(t[:], pt[:], b2b_s[:], ALU.add)
                        mx = wpool.tile([128, 1], F32, tag="mx")
                        nc.vector.tensor_reduce(mx[:], t[:], AX.X, ALU.max)
                        sh = wpool.tile([128, DOUT], F32, tag="sh")
                        nc.vector.tensor_scalar_sub(sh[:], t[:], mx[:])
                        ex = wpool.tile([128, DOUT], F32, tag="ex")
                        nc.scalar.activation(ex[:], sh[:], AF.Exp)
                        sm = wpool.tile([128, 1], F32, tag="sm")
                        nc.vector.tensor_reduce(sm[:], ex[:], AX.X, ALU.add)
                        lg = wpool.tile([128, 1], F32, tag="lg")
                        nc.scalar.activation(lg[:], sm[:], AF.Ln)
                        res = wpool.tile([128, DOUT], F16, tag="res")
                        nc.vector.tensor_scalar_sub(res[:], sh[:], lg[:])
                        r0 = (g * (GRP * WSLOT // 128) + k) * 128
                        nc.sync.dma_start(out_d[r0:r0 + 128, :], res[:])
    nc.compile()
    return nc


# ------------------------------------------------------- public entry
def kernel(x, edge_index, W1, b1, W2, b2, cfg=None, trace=False, time_reps=0):
    import time as _time

    from concourse.bass_utils import run_bass_kernel_spmd

    cfg = cfg or FULL
    N, NSH = cfg["N"], cfg["N"] // CORES
    DIN, DH, DOUT = cfg["DIN"], cfg["DH"], cfg["DOUT"]
    x = np.ascontiguousarray(np.asarray(x, dtype=np.float32))
    W1_h = np.asarray(W1, dtype=np.float32).astype(np.float16)
    b1_h = np.asarray(b1, dtype=np.float32)
    W2_h = np.asarray(W2, dtype=np.float32).astype(np.float16)
    b2_h = np.asarray(b2, dtype=np.float32)
    ident = np.eye(DOUT, dtype=np.float32)

    meta = preprocess(edge_index, cfg)
    c1, slots = meta["c1"], meta["slots"]

    def timed(nc, ins, store):
        res = run_bass_kernel_spmd(nc, ins, core_ids=list(range(CORES)),
                                   trace=trace)
        for _ in range(time_reps):
            t0 = _time.perf_counter()
            run_bass_kernel_spmd(nc, ins, core_ids=list(range(CORES)))
            store.append(_time.perf_counter() - t0)
        return res

    # ---- launch 0: xw1 = x @ W1 ----
    xq = x.astype(NP_F8)
    xT_in = [np.ascontiguousarray(xq[meta["nodes"][c]].T)
             for c in range(CORES)]
    nc_0 = build_nc_0(cfg, NSH)
    in_0 = [{"xT": xT_in[c], "W1": W1_h} for c in range(CORES)]
    kernel.times_0 = []
    res_0 = timed(nc_0, in_0, kernel.times_0)
    xw1_all = np.concatenate(
        [res_0.results[c]["xw1"] for c in range(CORES)], 0)

    def compact(table, refs):
        """Per-core table compaction: only rows this core references."""
        uniqs = [np.unique(refs[c].ravel(), return_inverse=True)
                 for c in range(CORES)]
        nt = max(len(u) for u, _ in uniqs)
        tabs, idxs = [], []
        for u, inv in uniqs:
            t = np.zeros((nt, table.shape[1]), table.dtype)
            t[:len(u)] = table[u]
            tabs.append(t)
            idxs.append(inv.reshape(CHUNK, -1).astype(np.int32))
        return tabs, idxs, nt

    # ---- launch A: layer 1 (device gathers xw1 rows per edge) ----
    ref1 = meta["rowpos"][meta["srcs"]]
    tab1, idx1, nt1 = compact(xw1_all, ref1)
    lane_iota = np.arange(CHUNK, dtype=np.float32)
    nc_a = build_nc_A(cfg, c1, nt1)
    in_a = [{"tab": tab1[c], "idx": idx1[c], "w": meta["wml"][c],
             "bnd": meta["bnd"][c], "iota": lane_iota,
             "W2": W2_h, "b1": b1_h} for c in range(CORES)]
    kernel.times_a = []
    res_a = timed(nc_a, in_a, kernel.times_a)

    # ---- host halo exchange ----
    xw2_all = np.concatenate(
        [res_a.results[c]["xw2"] for c in range(CORES)], 0)
    tab2, idx2, nt2 = compact(xw2_all, meta["pos_of"][meta["srcs"]])

    # ---- launch B: layer 2 (device gathers xw2 rows per edge) ----
    nc_b = build_nc_B(cfg, c1, nt2)
    in_b = [{"tab": tab2[c], "idx": idx2[c], "w": meta["wml"][c],
             "bnd": meta["bnd"][c], "iota": lane_iota,
             "b2": b2_h, "ident": ident} for c in range(CORES)]
    kernel.times_b = []
    res_b = timed(nc_b, in_b, kernel.times_b)

    out_full = np.zeros((N, DOUT), np.float32)
    for c in range(CORES):
        o = res_b.results[c]["out"].astype(np.float32)
        sel = meta["slot2node"][c] >= 0
        out_full[meta["slot2node"][c][sel]] = o[sel]
    return out_full


if __name__ == "__main__":
    cfg = dict(N=4096, E=65536, DIN=128, DH=64, DOUT=40)
    rng = np.random.default_rng(0)
    x = rng.normal(size=(cfg["N"], cfg["DIN"])).astype(np.float32)
    ei = rng.integers(0, cfg["N"], size=(2, cfg["E"])).astype(np.int64)
    W1 = (rng.normal(size=(cfg["DIN"], cfg["DH"])) / 16).astype(np.float32)
    b1 = (rng.normal(size=(cfg["DH"],)) * 0.1).astype(np.float32)
    W2 = (rng.normal(size=(cfg["DH"], cfg["DOUT"])) / 8).astype(np.float32)
    b2 = (rng.normal(size=(cfg["DOUT"],)) * 0.1).astype(np.float32)

    meta = preprocess(ei, cfg)
    print("c1:", meta["c1"], "slots:", meta["slots"],
          "pack_eff:", (cfg["E"] + cfg["N"]) / (meta["c1"] * CHUNK * CORES))
    got = emulate(x, W1, b1, W2, b2, meta, cfg)

    N = cfg["N"]
    loops = np.arange(N, dtype=np.int64)
    s = np.concatenate([ei[0], loops]); d = np.concatenate([ei[1], loops])
    deg = np.bincount(d, minlength=N).astype(np.float32)
    dis = np.where(deg > 0, 1 / np.sqrt(np.maximum(deg, 1)), 0).astype(np.float32)
    w = dis[s] * dis[d]

    def conv(xx, W, b):
        xw = xx @ W
        out = np.zeros((N, W.shape[1]), dtype=np.float32)
        np.add.at(out, d, xw[s] * w[:, None])
        return out + b

    h = np.maximum(conv(x, W1, b1), 0)
    o = conv(h, W2, b2)
    m = o.max(1, keepdims=True)
    ref = (o - m) - np.log(np.exp(o - m).sum(1, keepdims=True))
    err = np.abs(got - ref).max() / (np.abs(ref).max() + 1e-9)
    print("emulator vs ref max rel err:", err)
    assert err < 2e-3, err
    print("HOST LOGIC OK")


# revision 47
# speedup vs baseline: 2.1060x; 1.4570x over previous
"""2-layer GCN (gnn_message_passing) on 8 Trainium2 NeuronCores.

Strategy (graph/data parallel, dst-sharded, three SPMD launches):
  - Nodes sharded across 8 cores by destination id (12500 each). Host
    precomputes symmetric GCN normalization (graph preprocessing), adds
    self-loops, and bin-packs each core's nodes into uniform "chunks"
    (<=8 nodes, <=128 in-edges). Snake-balanced sharding (degree-sorted
    deal) plus a target-chasing packer with exact pair completion fill
    chunks to ~98% of the 128-lane capacity. One NEFF per stage runs
    SPMD on all 8 cores with per-core metadata tensors.
  - Per-edge random gather runs ON DEVICE via gpsimd indirect DMA
    (one offset per partition: each chunk's 128 lanes gather their
    source rows from a replicated fp8 feature table in one
    instruction). Each core therefore receives only the compact table
    (xw1: 6.4 MB, xw2: 4.3 MB) plus int32 per-lane row ids, instead of
    the ~2.4x-expanded per-edge feature streams.
  - The wall clock is dominated by host->device transfer (axon tunnel,
    ~36 MB/s serial), so every byte counts:
    * transform-first: launch 0 computes xw1 = x @ W1 on device, so
      the layer-1 table rows are 64 B fp8 instead of 512 B f32 raw x;
      the layer-2 table rows are 40 B fp8.
    * aggregation metadata per lane: int32 row id + fp16 norm weight
      (folded into the slot masks on device); per-chunk slot
      boundaries (9 fp16/chunk) expand to 0/1 masks via a DMA
      partition-broadcast + DVE is_le/sub, then scale by w.
    * end-to-end max rel err ~5e-3 (fp32 PSUM accumulation) vs the
      2e-2 gate.
  - Launch A: per-chunk gather + matmul msg^T @ mask -> feature-major
    PSUM, fused b1+ReLU (scalar engine), W2 matmul -> xw2 shard
    [slots, 40] fp8 (W2 matmul emits slot-major directly).
  - Host halo exchange: concatenates xw2 shards into the layer-2
    table; the device gathers it by source position in launch B.
  - Launch B: gather + aggregate the same way, PE-transpose to
    slot-major, add b2, log_softmax, write fp16 [slots, 40] per core.
  - Host un-permutes slot rows back to original node order.
"""

import numpy as np
import ml_dtypes

FULL = dict(N=100000, E=1600000, DIN=128, DH=64, DOUT=40)
CORES = 8
WSLOT = 8          # node slots per chunk
CHUNK = 128        # edge lanes per chunk
GRP = 16           # chunks per group  (GRP*WSLOT = 128 psum positions)
NP_F8 = ml_dtypes.float8_e4m3


# ------------------------------------------------------- host preprocessing
def _pack(degl):
    """Target-chasing bin-pack: <=WSLOT nodes, <=CHUNK edges per chunk.

    First item is the largest remaining degree; each further slot takes
    the available degree closest to cap/slots_left so chunks land near
    exactly CHUNK edges with ~WSLOT nodes (measured fill ~0.97).
    """
    n = len(degl)
    dmax = int(degl.max())
    by_deg = np.argsort(degl, kind="stable")
    startd = np.searchsorted(degl[by_deg], np.arange(dmax + 2))
    ptr = startd[1:].copy()              # pop position per degree bucket
    remaining = (startd[1:] - startd[:-1]).astype(np.int64)
    co = np.empty(n, np.int64)
    so = np.empty(n, np.int64)
    total, ci = n, 0
    while total > 0:
        # first: largest available
        d = dmax
        while d > 0 and remaining[d] == 0:
            d -= 1
        ptr[d] -= 1
        nl = by_deg[ptr[d]]
        remaining[d] -= 1
        total -= 1
        co[nl], so[nl] = ci, 0
        cap, k = CHUNK - d, 1
        while k < WSLOT and total > 0 and cap > 0:
            best, bestkey = 0, None
            if k == WSLOT - 2 and cap >= 2:
                # exact pair completion: pick d so that cap-d is also
                # available; the last slot then fills the chunk to CHUNK
                for dd in range(max(1, cap - dmax), min(cap - 1, dmax) + 1):
                    d2 = cap - dd
                    if d2 < 1 or d2 > dmax:
                        continue
                    if remaining[dd] >= (2 if d2 == dd else 1) \
                            and remaining[d2] >= 1:
                        key = (abs(dd - cap / 2), -dd)
                        if bestkey is None or key < bestkey:
                            bestkey, best = key, dd
            if best == 0:
                tgt = cap / (WSLOT - k)
                for d in range(1, min(cap, dmax) + 1):
                    if remaining[d] == 0:
                        continue
                    key = (abs(d - tgt), -d)
                    if bestkey is None or key < bestkey:
                        bestkey, best = key, d
            if best == 0:
                break
            ptr[best] -= 1
            nl = by_deg[ptr[best]]
            remaining[best] -= 1
            total -= 1
            co[nl], so[nl] = ci, k
            cap -= best
            k += 1
        ci += 1
    return co, so, ci


def preprocess(edge_index, cfg):
    """Graph preprocessing: norm weights, sharding, chunk packing.

    Returns per-core src ids / norm weight per edge lane
    ([CORES, CHUNK, c1]), per-chunk slot boundaries ([CORES, c1, 9]),
    slot maps, and the uniform chunk count c1.
    """
    N, NSH = cfg["N"], cfg["N"] // CORES
    src = np.asarray(edge_index[0], dtype=np.int64)
    dst = np.asarray(edge_index[1], dtype=np.int64)
    loops = np.arange(N, dtype=np.int64)
    s_all = np.concatenate([src, loops])
    d_all = np.concatenate([dst, loops])
    deg = np.bincount(d_all, minlength=N)
    dis = np.where(deg > 0, 1.0 / np.sqrt(np.maximum(deg, 1.0)), 0.0)
    dis = dis.astype(np.float32)

    o = np.argsort(d_all, kind="stable")
    s_srt, d_srt = s_all[o], d_all[o]
    w_srt = dis[s_srt] * dis[d_srt]
    seg = np.zeros(N + 1, np.int64)
    seg[1:] = np.cumsum(deg)

    # snake-balanced dst sharding: deal degree-sorted nodes across cores
    # so every core gets a near-identical degree multiset (equal packing)
    order = np.argsort(-deg, kind="stable")
    idx = np.arange(N)
    pos = idx % CORES
    core_rank = np.where((idx // CORES) % 2 == 0, pos, CORES - 1 - pos)
    nodes = [order[core_rank == c] for c in range(CORES)]

    packres = []
    nch = np.zeros(CORES, np.int64)
    for c in range(CORES):
        degl = deg[nodes[c]]
        assert degl.max() <= CHUNK, "node degree exceeds chunk capacity"
        assert degl.min() >= 1
        co, so, ncc = _pack(degl)
        packres.append((co, so))
        nch[c] = ncc

    c1 = ((int(nch.max()) + GRP - 1) // GRP) * GRP
    slots = c1 * WSLOT

    pos_of = np.empty(N, np.int64)
    rowpos = np.empty(N, np.int64)
    slot2node = np.full((CORES, slots), -1, np.int64)
    srcs = np.zeros((CORES, CHUNK, c1), np.int64)
    wml = np.zeros((CORES, CHUNK, c1), np.float16)
    bnd = np.zeros((CORES, c1, WSLOT + 1), np.float16)

    for c in range(CORES):
        ndc = nodes[c]
        co, so = packres[c]
        degl = deg[ndc]
        # lane base per node: exclusive cumsum of degrees in (chunk, slot) order
        ordk = np.argsort(co * WSLOT + so)
        degk = degl[ordk]
        cs = np.cumsum(degk) - degk
        cid = co[ordk]
        first = np.searchsorted(cid, np.arange(nch[c]), side="left")
        lane_base = np.empty(NSH, np.int64)
        lane_base[ordk] = cs - cs[first][cid]
        # ragged expansion of this core's edges (dst-sorted rows per node)
        lens = degl
        tot = int(lens.sum())
        cum = np.cumsum(lens) - lens
        within = np.arange(tot) - np.repeat(cum, lens)
        rows = np.repeat(seg[ndc], lens) + within
        eloc = np.repeat(np.arange(NSH), lens)
        lane_e = lane_base[eloc] + within
        assert lane_e.max() < CHUNK
        srcs[c, lane_e, co[eloc]] = s_srt[rows]
        wml[c, lane_e, co[eloc]] = w_srt[rows]
        pos_of[ndc] = c * slots + co * WSLOT + so
        rowpos[ndc] = c * NSH + np.arange(NSH)
        slot2node[c, co * WSLOT + so] = ndc
        # per-chunk slot boundaries: bnd[ci, s] = first lane of slot s,
        # bnd[ci, 8] = chunk fill; empty slots / pad chunks collapse to fill
        fill = np.zeros(c1, np.int64)
        np.add.at(fill, co, degl)
        bnd[c] = np.repeat(fill[:, None], WSLOT + 1, axis=1)
        bnd[c, co, so] = lane_base

    return dict(srcs=srcs, wml=wml, bnd=bnd, slot2node=slot2node,
                pos_of=pos_of, rowpos=rowpos, nodes=nodes, c1=c1, slots=slots)


# ------------------------------------------------------- numpy emulation
def emulate(x, W1, b1, W2, b2, meta, cfg):
    """Pure-numpy emulation of the device dataflow (logic validation)."""
    DOUT = cfg["DOUT"]
    c1, slots = meta["c1"], meta["slots"]
    srcs = meta["srcs"]
    lane = np.arange(CHUNK, dtype=np.float32)
    ge = meta["bnd"].astype(np.float32)[:, None, :, :] <= \
        lane[None, :, None, None]                 # [CORES, CHUNK, c1, 9]
    oh = ge[..., :WSLOT].astype(np.float32) - ge[..., 1:].astype(np.float32)
    wf = meta["wml"].astype(np.float32)[..., None]
    xw1 = x @ W1
    xw2_all = np.zeros((CORES * slots, DOUT), np.float32)
    for c in range(CORES):
        msg = wf[c] * xw1[srcs[c]]                # [CHUNK, c1, DH]
        hrawT = np.einsum("pcf,pcs->fcs", msg, oh[c]).reshape(-1, slots)
        hT = np.maximum(hrawT + b1[:, None], 0.0)
        xw2_all[c * slots:(c + 1) * slots] = (W2.T @ hT).T
    out_full = np.zeros((cfg["N"], DOUT), np.float32)
    for c in range(CORES):
        msg = wf[c] * xw2_all[meta["pos_of"][srcs[c]]]
        oT = np.einsum("pcf,pcs->fcs", msg, oh[c]).reshape(DOUT, slots)
        o = oT.T + b2[None, :]
        m = o.max(axis=1, keepdims=True)
        ls = (o - m) - np.log(np.exp(o - m).sum(axis=1, keepdims=True))
        sel = meta["slot2node"][c] >= 0
        out_full[meta["slot2node"][c][sel]] = ls[sel]
    return out_full


# ------------------------------------------------------- bass programs
def _bass_mods():
    import concourse.bass as bass
    import concourse.bacc as bacc
    import concourse.mybir as mybir
    import concourse.tile as tile
    return bass, bacc, mybir, tile


def _build_onehot(nc, tc, cpool, mybir, bnd_d, iota_d, w_d, c1):
    """Expand per-chunk slot boundaries to w-weighted masks [128, c1, 8].

    bnd[ci, s] is the first lane of slot s (bnd[ci, 8] = chunk fill);
    mask[lane, ci, s] = w[lane, ci] * (bnd[s] <= lane < bnd[s+1]). Pad
    lanes and pad chunks fall outside every [bnd[s], bnd[s+1]) interval
    and carry w = 0, so they are masked out structurally.
    """
    F16 = mybir.dt.float16
    F32 = mybir.dt.float32
    NB = WSLOT + 1
    ALU = mybir.AluOpType
    iota_s = cpool.tile([CHUNK, 1], F32)
    nc.sync.dma_start(iota_s[:], iota_d[:].unsqueeze(1))
    w_s = cpool.tile([CHUNK, c1], F16)
    nc.sync.dma_start(w_s[:], w_d[:, :])
    oh = cpool.tile([CHUNK, c1, WSLOT], F16)
    with tc.tile_pool(name="ohtmp", bufs=1) as tpool:
        bb = tpool.tile([CHUNK, c1, NB], F16)
        nc.sync.dma_start(
            bb[:], bnd_d[:, :].unsqueeze(0).broadcast_to([CHUNK, c1, NB]))
        ge = tpool.tile([CHUNK, c1, NB], F16)
        for s in range(NB):
            nc.vector.tensor_scalar(ge[:, :, s], bb[:, :, s], iota_s[:], None,
                                    ALU.is_le)
        mask = tpool.tile([CHUNK, c1], F16)
        for s in range(WSLOT):
            nc.vector.tensor_tensor(mask[:], ge[:, :, s], ge[:, :, s + 1],
                                    ALU.subtract)
            nc.vector.tensor_tensor(oh[:, :, s], mask[:], w_s[:], ALU.mult)
    return oh


def build_nc_0(cfg, nsh):
    """Launch 0: xw1 = x @ W1 per node shard (transform-first)."""
    bass, bacc, mybir, tile = _bass_mods()
    DIN, DH = cfg["DIN"], cfg["DH"]
    F8, F16, F32 = mybir.dt.float8e4, mybir.dt.float16, mybir.dt.float32
    PS = bass.MemorySpace.PSUM

    nc = bacc.Bacc(None, target_bir_lowering=False, num_devices=CORES)
    xT_d = nc.dram_tensor("xT", [DIN, nsh], F8, kind="ExternalInput")
    w1_d = nc.dram_tensor("W1", [DIN, DH], F16, kind="ExternalInput")
    xw1_d = nc.dram_tensor("xw1", [nsh, DH], F8, kind="ExternalOutput")

    with tile.TileContext(nc) as tc:
        with tc.tile_pool(name="const", bufs=1) as cpool:
            w1_s = cpool.tile([DIN, DH], F16)
            nc.sync.dma_start(w1_s[:], w1_d[:, :])
            xT_s = cpool.tile([DIN, nsh], F8)
            nc.sync.dma_start(xT_s[:], xT_d[:, :])
            with (
                tc.tile_pool(name="work", bufs=3) as wpool,
                tc.tile_pool(name="ps", bufs=4, space=PS) as pp,
            ):
                for t in range((nsh + 127) // 128):
                    n0 = t * 128
                    nn = min(128, nsh - n0)
                    p = pp.tile([128, DH], F32, tag="p")
                    nc.tensor.matmul(p[0:nn, :], xT_s[:, n0:n0 + nn],
                                     w1_s[:], start=True, stop=True)
                    ot = wpool.tile([128, DH], F8, tag="ot")
                    nc.vector.tensor_copy(ot[0:nn, :], p[0:nn, :])
                    nc.sync.dma_start(xw1_d[n0:n0 + nn, :], ot[0:nn, :])
    nc.compile()
    return nc


def build_nc_A(cfg, c1, nsh):
    """Launch A (fused): xw1 = x @ W1 per shard, on-device AllGather of
    the xw1 table, per-edge gather, layer-1 aggregation + b1/relu/W2."""
    bass, bacc, mybir, tile = _bass_mods()
    DIN, DH, DOUT = cfg["DIN"], cfg["DH"], cfg["DOUT"]
    F8, F16, F32 = mybir.dt.float8e4, mybir.dt.float16, mybir.dt.float32
    I32 = mybir.dt.int32
    AF = mybir.ActivationFunctionType
    ALU = mybir.AluOpType
    PS = bass.MemorySpace.PSUM
    slots, ng = c1 * WSLOT, c1 // GRP

    nc = bacc.Bacc(None, target_bir_lowering=False, num_devices=CORES)
    xT_d = nc.dram_tensor("xT", [DIN, nsh], F8, kind="ExternalInput")
    w1_d = nc.dram_tensor("W1", [DIN, DH], F16, kind="ExternalInput")
    idx_d = nc.dram_tensor("idx", [CHUNK, c1], I32, kind="ExternalInput")
    w_d = nc.dram_tensor("w", [CHUNK, c1], F16, kind="ExternalInput")
    bnd_d = nc.dram_tensor("bnd", [c1, WSLOT + 1], F16, kind="ExternalInput")
    iota_d = nc.dram_tensor("iota", [CHUNK], F32, kind="ExternalInput")
    w2_d = nc.dram_tensor("W2", [DH, DOUT], F16, kind="ExternalInput")
    b1_d = nc.dram_tensor("b1", [DH], F32, kind="ExternalInput")
    xw2_d = nc.dram_tensor("xw2", [slots, DOUT], F8, kind="ExternalOutput")
    xw1sh_i = nc.dram_tensor("xw1sh", [nsh, DH], F8, kind="Internal")
    xw1full_i = nc.dram_tensor("xw1full", [CORES * nsh, DH], F8,
                               kind="Internal")

    with tile.TileContext(nc) as tc:
        with tc.tile_pool(name="const", bufs=1) as cpool:
            w1_s = cpool.tile([DIN, DH], F16)
            nc.sync.dma_start(w1_s[:], w1_d[:, :])
            xT_s = cpool.tile([DIN, nsh], F8)
            nc.sync.dma_start(xT_s[:], xT_d[:, :])
            w2_s = cpool.tile([DH, DOUT], F16)
            nc.sync.dma_start(w2_s[:], w2_d[:, :])
            b1_s = cpool.tile([DH, 1], F32)
            nc.sync.dma_start(b1_s[:], b1_d[:].unsqueeze(1))
            idx_s = cpool.tile([CHUNK, c1], I32)
            nc.sync.dma_start(idx_s[:], idx_d[:, :])
            oh = _build_onehot(nc, tc, cpool, mybir, bnd_d, iota_d, w_d, c1)
            # prologue: this shard's xw1 rows -> internal DRAM
            with (
                tc.tile_pool(name="xwork", bufs=3) as xpool,
                tc.tile_pool(name="xps", bufs=4, space=PS) as xpp,
            ):
                for t in range((nsh + 127) // 128):
                    n0 = t * 128
                    nn = min(128, nsh - n0)
                    p = xpp.tile([128, DH], F32, tag="p")
                    nc.tensor.matmul(p[0:nn, :], xT_s[:, n0:n0 + nn],
                                     w1_s[:], start=True, stop=True)
                    ot = xpool.tile([128, DH], F8, tag="ot")
                    nc.vector.tensor_copy(ot[0:nn, :], p[0:nn, :])
                    nc.sync.dma_start(xw1sh_i[n0:n0 + nn, :], ot[0:nn, :])
            # halo exchange on device: full xw1 table in internal DRAM
            nc.gpsimd.collective_compute(
                "AllGather", ALU.bypass,
                replica_groups=[list(range(CORES))],
                ins=[xw1sh_i[:, :]], outs=[xw1full_i[:, :]])
            with (
                tc.tile_pool(name="gath", bufs=2) as gpool,
                tc.tile_pool(name="work", bufs=2) as wpool,
                tc.tile_pool(name="ps1", bufs=2, space=PS) as pp,
                tc.tile_pool(name="ps2", bufs=2, space=PS) as ppb,
            ):
                for g in range(ng):
                    msg = gpool.tile([CHUNK, GRP, DH], F8, tag="msg")
                    for c in range(GRP):
                        nc.gpsimd.indirect_dma_start(
                            out=msg[:, c, :], out_offset=None,
                            in_=xw1full_i[:],
                            in_offset=bass.IndirectOffsetOnAxis(
                                ap=idx_s[:, g * GRP + c:g * GRP + c + 1],
                                axis=0))
                    pg = pp.tile([DH, GRP * WSLOT], F32, tag="agg")
                    for c in range(GRP):
                        nc.tensor.matmul(
                            pg[:, c * WSLOT:(c + 1) * WSLOT],
                            msg[:, c, :], oh[:, g * GRP + c, :],
                            start=True, stop=True)
                    hT = wpool.tile([DH, GRP * WSLOT], F16, tag="hT")
                    nc.scalar.activation(hT[:], pg[:], AF.Relu, bias=b1_s[:])
                    for k in range(GRP * WSLOT // 128):
                        p2 = ppb.tile([128, DOUT], F32, tag="p2")
                        nc.tensor.matmul(p2[:], hT[:, k * 128:(k + 1) * 128],
                                         w2_s[:], start=True, stop=True)
                        ot = wpool.tile([128, DOUT], F8, tag="ot")
                        nc.vector.tensor_copy(ot[:], p2[:])
                        r0 = (g * (GRP * WSLOT // 128) + k) * 128
                        nc.sync.dma_start(xw2_d[r0:r0 + 128, :], ot[:])
    nc.compile()
    return nc


def build_nc_B(cfg, c1, ntab):
    """Launch B: gather xw2 rows, layer-2 aggregation + b2 + log_softmax."""
    bass, bacc, mybir, tile = _bass_mods()
    DOUT = cfg["DOUT"]
    F8, F16, F32 = mybir.dt.float8e4, mybir.dt.float16, mybir.dt.float32
    I32 = mybir.dt.int32
    AF = mybir.ActivationFunctionType
    ALU = mybir.AluOpType
    AX = mybir.AxisListType
    PS = bass.MemorySpace.PSUM
    slots, ng = c1 * WSLOT, c1 // GRP

    nc = bacc.Bacc(None, target_bir_lowering=False, num_devices=CORES)
    tab_d = nc.dram_tensor("tab", [ntab, DOUT], F8, kind="ExternalInput")
    idx_d = nc.dram_tensor("idx", [CHUNK, c1], I32, kind="ExternalInput")
    w_d = nc.dram_tensor("w", [CHUNK, c1], F16, kind="ExternalInput")
    bnd_d = nc.dram_tensor("bnd", [c1, WSLOT + 1], F16, kind="ExternalInput")
    iota_d = nc.dram_tensor("iota", [CHUNK], F32, kind="ExternalInput")
    b2_d = nc.dram_tensor("b2", [DOUT], F32, kind="ExternalInput")
    id_d = nc.dram_tensor("ident", [DOUT, DOUT], F32, kind="ExternalInput")
    out_d = nc.dram_tensor("out", [slots, DOUT], F16, kind="ExternalOutput")

    with tile.TileContext(nc) as tc:
        with tc.tile_pool(name="const", bufs=1) as cpool:
            id_s = cpool.tile([DOUT, DOUT], F32)
            nc.sync.dma_start(id_s[:], id_d[:, :])
            b2r_s = cpool.tile([1, DOUT], F32)
            nc.sync.dma_start(b2r_s[:], b2_d[:].unsqueeze(0))
            ones_s = cpool.tile([1, 128], F32)
            nc.vector.memset(ones_s[:], 1.0)
            b2b_s = cpool.tile([128, DOUT], F32)
            with tc.tile_pool(name="pbc", bufs=1, space=PS) as pbc:
                pb = pbc.tile([128, DOUT], F32)
                nc.tensor.matmul(pb[:], ones_s[:], b2r_s[:], start=True, stop=True)
                nc.vector.tensor_copy(b2b_s[:], pb[:])
            idx_s = cpool.tile([CHUNK, c1], I32)
            nc.sync.dma_start(idx_s[:], idx_d[:, :])
            oh = _build_onehot(nc, tc, cpool, mybir, bnd_d, iota_d, w_d, c1)
            with (
                tc.tile_pool(name="gath", bufs=2) as gpool,
                tc.tile_pool(name="work", bufs=2) as wpool,
                tc.tile_pool(name="ps1", bufs=2, space=PS) as pp,
                tc.tile_pool(name="ps2", bufs=2, space=PS) as ppb,
            ):
                for g in range(ng):
                    msg = gpool.tile([CHUNK, GRP, DOUT], F8, tag="msg")
                    for c in range(GRP):
                        nc.gpsimd.indirect_dma_start(
                            out=msg[:, c, :], out_offset=None, in_=tab_d[:],
                            in_offset=bass.IndirectOffsetOnAxis(
                                ap=idx_s[:, g * GRP + c:g * GRP + c + 1],
                                axis=0))
                    pg = pp.tile([DOUT, GRP * WSLOT], F32, tag="agg")
                    for c in range(GRP):
                        nc.tensor.matmul(
                            pg[:, c * WSLOT:(c + 1) * WSLOT],
                            msg[:, c, :], oh[:, g * GRP + c, :],
                            start=True, stop=True)
                    oT = wpool.tile([DOUT, GRP * WSLOT], F32, tag="oT")
                    nc.scalar.copy(oT[:], pg[:])
                    for k in range(GRP * WSLOT // 128):
                        pt = ppb.tile([128, DOUT], F32, tag="pt")
                        nc.tensor.transpose(pt[:], oT[:, k * 128:(k + 1) * 128],
                                            id_s[:])
                        t = wpool.tile([128, DOUT], F32, tag="t")
                        nc.vector.tensor_tensor(t[:], pt[:], b2b_s[:], ALU.add)
                        mx = wpool.tile([128, 1], F32, tag="mx")
                        nc.vector.tensor_reduce(mx[:], t[:], AX.X, ALU.max)
                        sh = wpool.tile([128, DOUT], F32, tag="sh")
                        nc.vector.tensor_scalar_sub(sh[:], t[:], mx[:])
                        ex = wpool.tile([128, DOUT], F32, tag="ex")
                        nc.scalar.activation(ex[:], sh[:], AF.Exp)
                        sm = wpool.tile([128, 1], F32, tag="sm")
                        nc.vector.tensor_reduce(sm[:], ex[:], AX.X, ALU.add)
                        lg = wpool.tile([128, 1], F32, tag="lg")
                        nc.scalar.activation(lg[:], sm[:], AF.Ln)
                        res = wpool.tile([128, DOUT], F16, tag="res")
                        nc.vector.tensor_scalar_sub(res[:], sh[:], lg[:])
                        r0 = (g * (GRP * WSLOT // 128) + k) * 128
                        nc.sync.dma_start(out_d[r0:r0 + 128, :], res[:])
    nc.compile()
    return nc


# ------------------------------------------------------- public entry
def kernel(x, edge_index, W1, b1, W2, b2, cfg=None, trace=False, time_reps=0):
    import time as _time

    from concourse.bass_utils import run_bass_kernel_spmd

    cfg = cfg or FULL
    N, NSH = cfg["N"], cfg["N"] // CORES
    DIN, DH, DOUT = cfg["DIN"], cfg["DH"], cfg["DOUT"]
    x = np.ascontiguousarray(np.asarray(x, dtype=np.float32))
    W1_h = np.asarray(W1, dtype=np.float32).astype(np.float16)
    b1_h = np.asarray(b1, dtype=np.float32)
    W2_h = np.asarray(W2, dtype=np.float32).astype(np.float16)
    b2_h = np.asarray(b2, dtype=np.float32)
    ident = np.eye(DOUT, dtype=np.float32)

    meta = preprocess(edge_index, cfg)
    c1, slots = meta["c1"], meta["slots"]

    def timed(nc, ins, store):
        res = run_bass_kernel_spmd(nc, ins, core_ids=list(range(CORES)),
                                   trace=trace)
        for _ in range(time_reps):
            t0 = _time.perf_counter()
            run_bass_kernel_spmd(nc, ins, core_ids=list(range(CORES)))
            store.append(_time.perf_counter() - t0)
        return res

    xq = x.astype(NP_F8)
    xT_in = [np.ascontiguousarray(xq[meta["nodes"][c]].T)
             for c in range(CORES)]
    kernel.times_0 = []

    def compact(table, refs):
        """Per-core table compaction: only rows this core references."""
        uniqs = [np.unique(refs[c].ravel(), return_inverse=True)
                 for c in range(CORES)]
        nt = max(len(u) for u, _ in uniqs)
        tabs, idxs = [], []
        for u, inv in uniqs:
            t = np.zeros((nt, table.shape[1]), table.dtype)
            t[:len(u)] = table[u]
            tabs.append(t)
            idxs.append(inv.reshape(CHUNK, -1).astype(np.int32))
        return tabs, idxs, nt

    # ---- launch A (fused): xw1 on device + AllGather + gather + layer 1
    ref1 = meta["rowpos"][meta["srcs"]].astype(np.int32)
    lane_iota = np.arange(CHUNK, dtype=np.float32)
    nc_a = build_nc_A(cfg, c1, NSH)
    in_a = [{"xT": xT_in[c], "W1": W1_h,
             "idx": ref1[c], "w": meta["wml"][c],
             "bnd": meta["bnd"][c], "iota": lane_iota,
             "W2": W2_h, "b1": b1_h} for c in range(CORES)]
    kernel.times_a = []
    res_a = timed(nc_a, in_a, kernel.times_a)

    # ---- host halo exchange ----
    xw2_all = np.concatenate(
        [res_a.results[c]["xw2"] for c in range(CORES)], 0)
    tab2, idx2, nt2 = compact(xw2_all, meta["pos_of"][meta["srcs"]])

    # ---- launch B: layer 2 (device gathers xw2 rows per edge) ----
    nc_b = build_nc_B(cfg, c1, nt2)
    in_b = [{"tab": tab2[c], "idx": idx2[c], "w": meta["wml"][c],
             "bnd": meta["bnd"][c], "iota": lane_iota,
             "b2": b2_h, "ident": ident} for c in range(CORES)]
    kernel.times_b = []
    res_b = timed(nc_b, in_b, kernel.times_b)

    out_full = np.zeros((N, DOUT), np.float32)
    for c in range(CORES):
        o = res_b.results[c]["out"].astype(np.float32)
        sel = meta["slot2node"][c] >= 0
        out_full[meta["slot2node"][c][sel]] = o[sel]
    return out_full


if __name__ == "__main__":
    cfg = dict(N=4096, E=65536, DIN=128, DH=64, DOUT=40)
    rng = np.random.default_rng(0)
    x = rng.normal(size=(cfg["N"], cfg["DIN"])).astype(np.float32)
    ei = rng.integers(0, cfg["N"], size=(2, cfg["E"])).astype(np.int64)
    W1 = (rng.normal(size=(cfg["DIN"], cfg["DH"])) / 16).astype(np.float32)
    b1 = (rng.normal(size=(cfg["DH"],)) * 0.1).astype(np.float32)
    W2 = (rng.normal(size=(cfg["DH"], cfg["DOUT"])) / 8).astype(np.float32)
    b2 = (rng.normal(size=(cfg["DOUT"],)) * 0.1).astype(np.float32)

    meta = preprocess(ei, cfg)
    print("c1:", meta["c1"], "slots:", meta["slots"],
          "pack_eff:", (cfg["E"] + cfg["N"]) / (meta["c1"] * CHUNK * CORES))
    got = emulate(x, W1, b1, W2, b2, meta, cfg)

    N = cfg["N"]
    loops = np.arange(N, dtype=np.int64)
    s = np.concatenate([ei[0], loops]); d = np.concatenate([ei[1], loops])
    deg = np.bincount(d, minlength=N).astype(np.float32)
    dis = np.where(deg > 0, 1 / np.sqrt(np.maximum(deg, 1)), 0).astype(np.float32)
    w = dis[s] * dis[d]

    def conv(xx, W, b):
        xw = xx @ W
        out = np.zeros((N, W.shape[1]), dtype=np.float32)
        np.add.at(out, d, xw[s] * w[:, None])
        return out + b

    h = np.maximum(conv(x, W1, b1), 0)
    o = conv(h, W2, b2)
    m = o.max(1, keepdims=True)
    ref = (o - m) - np.log(np.exp(o - m).sum(1, keepdims=True))
    err = np.abs(got - ref).max() / (np.abs(ref).max() + 1e-9)
    print("emulator vs ref max rel err:", err)
    assert err < 2e-3, err
    print("HOST LOGIC OK")
